# revision 13
# baseline (speedup 1.0000x reference)
"""Bass/Trainium2 kernel for nn_DeepIRTModel (DKVMN knowledge tracing).

Strategy: data-parallel over batch (B=256 -> 32 per core on 8 cores).
Per core, sample index r = 32*t + b (t-major). Scan state X = Mv lives in
SBUF as (128 partitions = 32*g+b, free = (j,d)) with slot n = 13*g + j
(52 padded slots, pad weights = 0), d = DV = 200.

Per scan step t (engine assignment):
  DVE:  V = X * er_bcast ; V -= ad_bcast ; wG slices j>=KACT ; X -= wG ;
        W_sel = w_slab_bcast * I_sel_bcast
  ACT:  wG slices j < KACT ; readT PSUM->SBUF copies
  PE:   readT += X_dslice^T @ W_sel_j  (26 matmuls, PSUM accum over j)
Reads use X BEFORE the update (Tile's WAR deps order PE before the X write).
theta/beta/pred are batched matmul chains after the scan.
"""
import sys, types

sys.path.insert(0, '/opt/trn_rl_repo')
import numpy as np
import ml_dtypes


def _install_ntff_hook():
    try:
        import antenv
        if "antenv.axon_hooks" in sys.modules:
            return
        mod = types.ModuleType("antenv.axon_hooks")
        state = {"hook": None}
        mod.set_axon_ntff_profile_hook = lambda h: state.__setitem__("hook", h)
        mod.get_axon_ntff_profile_hook = lambda: state["hook"]
        sys.modules["antenv.axon_hooks"] = mod
        antenv.axon_hooks = mod
        from trn_agent_boot.trn_boot import _ntff_profile_via_ctypes
        mod.set_axon_ntff_profile_hook(_ntff_profile_via_ctypes('/opt/axon/libaxon_pjrt.so'))
    except Exception:
        pass


_install_ntff_hook()

import concourse.bass as bass
import concourse.bacc as bacc
import concourse.mybir as mybir
from concourse.tile import TileContext, add_dep_helper
from concourse.bass_utils import run_bass_kernel_spmd

BF16 = mybir.dt.bfloat16
F32 = mybir.dt.float32
I32 = mybir.dt.int32
AF = mybir.ActivationFunctionType
OP = mybir.AluOpType
AX = mybir.AxisListType

NQ, M, DK, DV, DS = 50000, 50, 50, 200, 50
B, T_FULL = 256, 200
SCALE = 3.0
NCORES = 8
BL = B // NCORES          # 32 batch rows per core
NJ = 13                   # slot groups per partition-subindex g in [0,4)
MP = 4 * NJ               # 52 padded slots
FREE = NJ * DV            # 2600 state free dim
KACT = 6                  # wG slices computed on ACT (j < KACT)


def build_nc(T=T_FULL, Tc=25, sim_safe=False):
    R = BL * T            # samples per core
    NCH = R // 128        # gather chunks of 128 rows
    assert R % 128 == 0
    NSEG = (R + 511) // 512
    nchunks = (T + Tc - 1) // Tc

    nc = bacc.Bacc(trn_type="TRN2")
    # ---- DRAM I/O ----
    qi_d = nc.dram_tensor("qi", [R], I32, kind="ExternalInput")
    qai_d = nc.dram_tensor("qai", [R], I32, kind="ExternalInput")
    qtab_d = nc.dram_tensor("qtab", [NQ + 1, DK], F32, kind="ExternalInput")
    qatab_d = nc.dram_tensor("qatab", [2 * NQ + 1, DV], F32, kind="ExternalInput")
    x0_d = nc.dram_tensor("x0", [128, FREE], BF16, kind="ExternalInput")
    isel_d = nc.dram_tensor("isel", [128, BL], BF16, kind="ExternalInput")
    ident_d = nc.dram_tensor("ident", [128, 128], F32, kind="ExternalInput")
    mkt_d = nc.dram_tensor("mkt", [DK, M], BF16, kind="ExternalInput")
    we1_d = nc.dram_tensor("we1", [128, DV], BF16, kind="ExternalInput")
    we2_d = nc.dram_tensor("we2", [73, DV], BF16, kind="ExternalInput")
    wad1_d = nc.dram_tensor("wad1", [128, DV], BF16, kind="ExternalInput")
    wad2_d = nc.dram_tensor("wad2", [73, DV], BF16, kind="ExternalInput")
    wd1_d = nc.dram_tensor("wd1", [DK + 1, DS], BF16, kind="ExternalInput")
    wd2_d = nc.dram_tensor("wd2", [DS + 1, 1], BF16, kind="ExternalInput")
    ws1_d = nc.dram_tensor("ws1", [128, DS], BF16, kind="ExternalInput")
    ws2_d = nc.dram_tensor("ws2", [72, DS], BF16, kind="ExternalInput")
    ws3_d = nc.dram_tensor("ws3", [DK + 1, DS], BF16, kind="ExternalInput")
    wa1_d = nc.dram_tensor("wa1", [DS + 1, DS], BF16, kind="ExternalInput")
    wa2_d = nc.dram_tensor("wa2", [DS + 1, 1], BF16, kind="ExternalInput")
    ones_d = nc.dram_tensor("ones", [1, BL * T], BF16, kind="ExternalInput")
    pred_d = nc.dram_tensor("pred", [BL, T], F32, kind="ExternalOutput")
    # internal DRAM roundtrip buffers
    er_dram = nc.dram_tensor("er_dram", [R, DV], BF16)
    ad_dram = nc.dram_tensor("ad_dram", [R, DV], BF16)
    w_dram = nc.dram_tensor("w_dram", [R, MP], F32)

    with TileContext(nc) as tc:
        with tc.tile_pool(name="pers", bufs=1) as pers, \
             tc.tile_pool(name="work", bufs=2) as work:

            # ---------- phase A: params + indices ----------
            ident = pers.tile([128, 128], F32)
            nc.sync.dma_start(out=ident, in_=ident_d[:, :])
            iselb = pers.tile([128, BL], BF16)
            nc.sync.dma_start(out=iselb, in_=isel_d[:, :])
            x = pers.tile([128, FREE], BF16)
            nc.sync.dma_start(out=x, in_=x0_d[:, :])
            mkt = pers.tile([DK, M], BF16)
            nc.sync.dma_start(out=mkt, in_=mkt_d[:, :])
            wd1 = pers.tile([DK + 1, DS], BF16)
            nc.sync.dma_start(out=wd1, in_=wd1_d[:, :])
            wd2 = pers.tile([DS + 1, 1], BF16)
            nc.sync.dma_start(out=wd2, in_=wd2_d[:, :])
            ws1 = pers.tile([128, DS], BF16)
            nc.sync.dma_start(out=ws1, in_=ws1_d[:, :])
            ws2 = pers.tile([72, DS], BF16)
            nc.sync.dma_start(out=ws2, in_=ws2_d[:, :])
            ws3 = pers.tile([DK + 1, DS], BF16)
            nc.sync.dma_start(out=ws3, in_=ws3_d[:, :])
            wa1 = pers.tile([DS + 1, DS], BF16)
            nc.sync.dma_start(out=wa1, in_=wa1_d[:, :])
            wa2 = pers.tile([DS + 1, 1], BF16)
            nc.sync.dma_start(out=wa2, in_=wa2_d[:, :])

            qi_sb = pers.tile([128, NCH], I32)
            nc.sync.dma_start(out=qi_sb, in_=qi_d.rearrange("(k p) -> p k", p=128))
            qai_sb = pers.tile([128, NCH], I32)
            nc.sync.dma_start(out=qai_sb, in_=qai_d.rearrange("(k p) -> p k", p=128))

            # persistent across phases
            qeT = pers.tile([DK + 1, R], BF16)
            nc.sync.dma_start(out=qeT[DK:DK + 1, :], in_=ones_d[:, :R])
            beta_sb = pers.tile([128, NCH], F32)
            w_scan = pers.tile([128, T * NJ], F32)
            w_scanb = pers.tile([128, T * NJ], BF16)
            readT_lo = pers.tile([128, R], BF16)
            readT_hi = pers.tile([72, R], BF16)
            zpad = pers.tile([128, 2], F32)
            nc.vector.memset(zpad, 0.0)

            # ---------- phases A2-C: gather, transpose, w/er/ad/beta ----------
            with tc.tile_pool(name="ac_sb", bufs=1) as ac_sb, \
                 tc.tile_pool(name="ac_ps", bufs=2, space="PSUM") as ac_ps:
                we1 = ac_sb.tile([128, DV], BF16)
                nc.sync.dma_start(out=we1, in_=we1_d[:, :])
                we2 = ac_sb.tile([73, DV], BF16)
                nc.sync.dma_start(out=we2, in_=we2_d[:, :])
                wad1 = ac_sb.tile([128, DV], BF16)
                nc.sync.dma_start(out=wad1, in_=wad1_d[:, :])
                wad2 = ac_sb.tile([73, DV], BF16)
                nc.sync.dma_start(out=wad2, in_=wad2_d[:, :])
                qaeT_lo = ac_sb.tile([128, R], BF16)
                qaeT_hi = ac_sb.tile([73, R], BF16)
                nc.sync.dma_start(out=qaeT_hi[72:73, :], in_=ones_d[:, :R])
                h2T = ac_sb.tile([DS + 1, R], BF16)
                nc.sync.dma_start(out=h2T[DS:DS + 1, :], in_=ones_d[:, :R])

                w_w_insts = []
                er_w_insts = {}
                ad_w_insts = {}
                for k in range(NCH):
                    qe_g = ac_sb.tile([128, DK], F32, tag="qe_g", bufs=3)
                    nc.gpsimd.indirect_dma_start(
                        out=qe_g, out_offset=None, in_=qtab_d[:, :],
                        in_offset=bass.IndirectOffsetOnAxis(ap=qi_sb[:, k:k + 1], axis=0))
                    qae_g = ac_sb.tile([128, DV], F32, tag="qae_g", bufs=3)
                    nc.gpsimd.indirect_dma_start(
                        out=qae_g, out_offset=None, in_=qatab_d[:, :],
                        in_offset=bass.IndirectOffsetOnAxis(ap=qai_sb[:, k:k + 1], axis=0))
                    pt = ac_ps.tile([128, 128], F32, tag="pt", bufs=2)
                    nc.tensor.transpose(out=pt[:DK, :], in_=qe_g, identity=ident)
                    nc.scalar.copy(qeT[:DK, 128 * k:128 * (k + 1)], pt[:DK, :])
                    pt2 = ac_ps.tile([128, 128], F32, tag="pt", bufs=2)
                    nc.tensor.transpose(out=pt2, in_=qae_g[:, :128], identity=ident)
                    nc.scalar.copy(qaeT_lo[:, 128 * k:128 * (k + 1)], pt2)
                    pt3 = ac_ps.tile([128, 128], F32, tag="pt", bufs=2)
                    nc.tensor.transpose(out=pt3[:72, :], in_=qae_g[:, 128:200], identity=ident)
                    nc.scalar.copy(qaeT_hi[:72, 128 * k:128 * (k + 1)], pt3[:72, :])

                for k in range(NCH):
                    lg = ac_ps.tile([128, M], F32, tag="lg", bufs=2)
                    nc.tensor.matmul(lg, lhsT=qeT[:DK, 128 * k:128 * (k + 1)], rhs=mkt,
                                     start=True, stop=True)
                    ex = ac_sb.tile([128, M], F32, tag="ex", bufs=3)
                    nc.scalar.activation(out=ex, in_=lg, func=AF.Exp)
                    sm = ac_sb.tile([128, 1], F32, tag="sm", bufs=3)
                    nc.vector.reduce_sum(sm, ex, axis=AX.X)
                    rc = ac_sb.tile([128, 1], F32, tag="rc", bufs=3)
                    nc.vector.reciprocal(rc, sm)
                    wn = ac_sb.tile([128, M], F32, tag="wn", bufs=3)
                    nc.vector.tensor_scalar(out=wn, in0=ex, scalar1=rc[:, 0:1],
                                            scalar2=None, op0=OP.mult)
                    w_w_insts.append(nc.sync.dma_start(
                        out=w_dram[128 * k:128 * (k + 1), :M], in_=wn))
                    w_w_insts.append(nc.sync.dma_start(
                        out=w_dram[128 * k:128 * (k + 1), M:MP], in_=zpad[:, :2]))

                    ep = ac_ps.tile([128, DV], F32, tag="eap", bufs=2, name="ep")
                    nc.tensor.matmul(ep, lhsT=qaeT_lo[:, 128 * k:128 * (k + 1)], rhs=we1,
                                     start=True, stop=False)
                    nc.tensor.matmul(ep, lhsT=qaeT_hi[:, 128 * k:128 * (k + 1)], rhs=we2,
                                     start=False, stop=True)
                    ero = ac_sb.tile([128, DV], BF16, tag="ero", bufs=3)
                    nc.scalar.activation(out=ero, in_=ep, func=AF.Sigmoid)
                    er_w_insts[k] = nc.sync.dma_start(
                        out=er_dram[128 * k:128 * (k + 1), :], in_=ero)

                    ap_ = ac_ps.tile([128, DV], F32, tag="eap", bufs=2, name="ap_")
                    nc.tensor.matmul(ap_, lhsT=qaeT_lo[:, 128 * k:128 * (k + 1)], rhs=wad1,
                                     start=True, stop=False)
                    nc.tensor.matmul(ap_, lhsT=qaeT_hi[:, 128 * k:128 * (k + 1)], rhs=wad2,
                                     start=False, stop=True)
                    ado = ac_sb.tile([128, DV], BF16, tag="ado", bufs=3)
                    nc.scalar.activation(out=ado, in_=ap_, func=AF.Tanh)
                    ad_w_insts[k] = nc.sync.dma_start(
                        out=ad_dram[128 * k:128 * (k + 1), :], in_=ado)

                # beta chain
                for s in range(NSEG):
                    n0, n1 = 512 * s, min(512 * (s + 1), R)
                    hp = ac_ps.tile([DS, 512], F32, tag="hp", bufs=1)
                    nc.tensor.matmul(hp[:, :n1 - n0], lhsT=wd1, rhs=qeT[:, n0:n1],
                                     start=True, stop=True)
                    nc.scalar.activation(out=h2T[:DS, n0:n1], in_=hp[:, :n1 - n0],
                                         func=AF.Tanh)
                beta_ps = ac_ps.tile([128, NCH], F32, tag="beta_ps", bufs=1)
                for k in range(NCH):
                    nc.tensor.matmul(beta_ps[:, k:k + 1],
                                     lhsT=h2T[:, 128 * k:128 * (k + 1)],
                                     rhs=wd2, start=True, stop=True)
                nc.scalar.copy(beta_sb, beta_ps)

                # w_scan load: (128=(4b+g), (t,13)) from w_dram[(32t+b), 13g+j]
                if sim_safe:
                    nc.vector.memset(w_scan, 0.0)
                for g in range(4):
                    src = w_dram[:, NJ * g:NJ * (g + 1)].rearrange(
                        "(t b) j -> b t j", b=BL)
                    dst = w_scan[32 * g:32 * (g + 1), :].rearrange("b (t j) -> b t j", j=NJ)
                    ld = nc.sync.dma_start(out=dst, in_=src)
                    for wi in w_w_insts:
                        add_dep_helper(ld.ins, wi.ins, reason="w_dram roundtrip")
                nc.vector.tensor_copy(w_scanb, w_scan)

            # ---------- phase D: the scan ----------
            er_src = er_dram.rearrange("(t b) d -> t b d", b=BL)
            ad_src = ad_dram.rearrange("(t b) d -> t b d", b=BL)
            with tc.tile_pool(name="d_ps", bufs=2, space="PSUM") as d_ps:
                for ch in range(nchunks):
                    t0, t1 = Tc * ch, min(Tc * (ch + 1), T)
                    tl = t1 - t0
                    er_ch = work.tile([128, Tc * DV], BF16, tag="er_ch", bufs=2)
                    ad_ch = work.tile([128, Tc * DV], BF16, tag="ad_ch", bufs=2)
                    if sim_safe:
                        nc.vector.memset(er_ch, 0.0)
                        nc.vector.memset(ad_ch, 0.0)
                    ks = range(t0 // 4, (t1 + 3) // 4)
                    for g in range(4):
                        ldi = nc.sync.dma_start(
                            out=er_ch[32 * g:32 * (g + 1), :tl * DV].rearrange(
                                "b (t d) -> b t d", d=DV),
                            in_=er_src[t0:t1, :, :].rearrange("t b d -> b t d"))
                        for k in ks:
                            add_dep_helper(ldi.ins, er_w_insts[k].ins, reason="er roundtrip")
                        lda = nc.sync.dma_start(
                            out=ad_ch[32 * g:32 * (g + 1), :tl * DV].rearrange(
                                "b (t d) -> b t d", d=DV),
                            in_=ad_src[t0:t1, :, :].rearrange("t b d -> b t d"))
                        for k in ks:
                            add_dep_helper(lda.ins, ad_w_insts[k].ins, reason="ad roundtrip")

                    for tt in range(tl):
                        t = t0 + tt
                        er_t = er_ch[:, tt * DV:(tt + 1) * DV]
                        ad_t = ad_ch[:, tt * DV:(tt + 1) * DV]
                        er_b = er_t.rearrange("p (o d) -> p o d", o=1).to_broadcast(
                            [128, NJ, DV])
                        ad_b = ad_t.rearrange("p (o d) -> p o d", o=1).to_broadcast(
                            [128, NJ, DV])

                        wsel = work.tile([128, NJ * BL], BF16, tag="wsel", bufs=2)
                        nc.vector.tensor_tensor(
                            out=wsel.rearrange("p (j b) -> p j b", j=NJ),
                            in0=w_scanb[:, t * NJ:(t + 1) * NJ].rearrange(
                                "p (j o) -> p j o", o=1).to_broadcast([128, NJ, BL]),
                            in1=iselb.rearrange("p (o b) -> p o b", o=1).to_broadcast(
                                [128, NJ, BL]),
                            op=OP.mult)

                        # PE: readT (uses X before this step's update)
                        rlo = d_ps.tile([128, BL], F32, tag="rlo", bufs=2)
                        rhi = d_ps.tile([72, BL], F32, tag="rhi", bufs=2)
                        for j in range(NJ):
                            nc.tensor.matmul(rlo, lhsT=x[:, j * DV:j * DV + 128],
                                             rhs=wsel[:, j * BL:(j + 1) * BL],
                                             start=(j == 0), stop=(j == NJ - 1))
                        for j in range(NJ):
                            nc.tensor.matmul(rhi, lhsT=x[:, j * DV + 128:(j + 1) * DV],
                                             rhs=wsel[:, j * BL:(j + 1) * BL],
                                             start=(j == 0), stop=(j == NJ - 1))
                        nc.scalar.copy(readT_lo[:, BL * t:BL * (t + 1)], rlo)
                        nc.scalar.copy(readT_hi[:, BL * t:BL * (t + 1)], rhi[:72, :])

                        # DVE: V = X*er_b ; V -= ad_b
                        v = work.tile([128, FREE], BF16, tag="v", bufs=2)
                        v3 = v.rearrange("p (j d) -> p j d", j=NJ)
                        nc.vector.tensor_tensor(out=v3, in0=x.rearrange(
                            "p (j d) -> p j d", j=NJ), in1=er_b, op=OP.mult)
                        nc.vector.tensor_tensor(out=v3, in0=v3, in1=ad_b, op=OP.subtract)
                        wg = work.tile([128, FREE], BF16, tag="wg", bufs=2)
                        for j in range(NJ):
                            wcol = w_scan[:, t * NJ + j:t * NJ + j + 1]
                            if j < KACT:
                                nc.scalar.activation(out=wg[:, j * DV:(j + 1) * DV],
                                                     in_=v[:, j * DV:(j + 1) * DV],
                                                     func=AF.Copy, scale=wcol)
                            else:
                                nc.vector.tensor_scalar(out=wg[:, j * DV:(j + 1) * DV],
                                                        in0=v[:, j * DV:(j + 1) * DV],
                                                        scalar1=wcol, scalar2=None,
                                                        op0=OP.mult)
                        nc.vector.tensor_tensor(out=x, in0=x, in1=wg, op=OP.subtract)

            # ---------- phase E: summ/theta/pred ----------
            with tc.tile_pool(name="e_ps", bufs=2, space="PSUM") as e_ps:
                summT = pers.tile([DS + 1, R], BF16)
                nc.sync.dma_start(out=summT[DS:DS + 1, :], in_=ones_d[:, :R])
                for s in range(NSEG):
                    n0, n1 = 512 * s, min(512 * (s + 1), R)
                    sp = e_ps.tile([DS, 512], F32, tag="sp", bufs=2)
                    nc.tensor.matmul(sp[:, :n1 - n0], lhsT=ws1, rhs=readT_lo[:, n0:n1],
                                     start=True, stop=False)
                    nc.tensor.matmul(sp[:, :n1 - n0], lhsT=ws2, rhs=readT_hi[:72, n0:n1],
                                     start=False, stop=False)
                    nc.tensor.matmul(sp[:, :n1 - n0], lhsT=ws3, rhs=qeT[:, n0:n1],
                                     start=False, stop=True)
                    nc.scalar.activation(out=summT[:DS, n0:n1], in_=sp[:, :n1 - n0],
                                         func=AF.Tanh)
                hT = pers.tile([DS + 1, R], BF16)
                nc.sync.dma_start(out=hT[DS:DS + 1, :], in_=ones_d[:, :R])
                for s in range(NSEG):
                    n0, n1 = 512 * s, min(512 * (s + 1), R)
                    hp2 = e_ps.tile([DS, 512], F32, tag="hp2", bufs=2)
                    nc.tensor.matmul(hp2[:, :n1 - n0], lhsT=wa1, rhs=summT[:, n0:n1],
                                     start=True, stop=True)
                    nc.scalar.activation(out=hT[:DS, n0:n1], in_=hp2[:, :n1 - n0],
                                         func=AF.Tanh)
                th_ps = e_ps.tile([128, NCH], F32, tag="th_ps", bufs=1)
                for k in range(NCH):
                    nc.tensor.matmul(th_ps[:, k:k + 1], lhsT=hT[:, 128 * k:128 * (k + 1)],
                                     rhs=wa2, start=True, stop=True)
                pre = pers.tile([128, NCH], F32)
                nc.vector.scalar_tensor_tensor(out=pre, in0=th_ps, scalar=SCALE,
                                               in1=beta_sb, op0=OP.mult, op1=OP.subtract)
                pred_sb = pers.tile([128, NCH], F32)
                nc.scalar.activation(out=pred_sb, in_=pre, func=AF.Sigmoid)
                # pred_sb[p=32u+i, k] -> pred_d[b=i, t=4k+u]
                nc.sync.dma_start(
                    out=pred_d.rearrange("b (k u) -> u b k", u=4),
                    in_=pred_sb[:, :])

    return nc


_NC_CACHE = {}


def _get_nc(T=T_FULL):
    if T not in _NC_CACHE:
        n = build_nc(T=T)
        n.compile()
        _NC_CACHE[T] = n
    return _NC_CACHE[T]


def make_inmaps(q_data, qa_data, q_tab, qa_tab, Mk, Mv0, Ws, bs, Wa1, ba1, Wa2,
                ba2, Wd1, bd1, Wd2, bd2, We, be, Wad, bad, T=T_FULL):
    bf = ml_dtypes.bfloat16
    f32 = np.float32
    q_data = np.asarray(q_data)
    qa_data = np.asarray(qa_data)

    mv0p = np.zeros((MP, DV), f32)
    mv0p[:M] = np.asarray(Mv0, f32)
    x0 = mv0p.reshape(4, NJ, DV)[:, None].repeat(BL, 1).reshape(128, FREE).astype(bf)
    isel = np.tile(np.eye(BL, dtype=f32), (4, 1)).astype(bf)  # (128, 32), g-major
    ident = np.eye(128, dtype=f32)

    cat = np.concatenate
    common = {
        "qtab": np.ascontiguousarray(np.asarray(q_tab, f32)),
        "qatab": np.ascontiguousarray(np.asarray(qa_tab, f32)),
        "x0": x0, "isel": isel, "ident": ident,
        "ones": np.ones((1, BL * T), f32).astype(bf),
        "mkt": np.asarray(Mk, f32).T.copy().astype(bf),
        "we1": np.asarray(We, f32)[:128].astype(bf),
        "we2": cat([np.asarray(We, f32)[128:], np.asarray(be, f32)[None, :]], 0).astype(bf),
        "wad1": np.asarray(Wad, f32)[:128].astype(bf),
        "wad2": cat([np.asarray(Wad, f32)[128:], np.asarray(bad, f32)[None, :]], 0).astype(bf),
        "wd1": cat([np.asarray(Wd1, f32), np.asarray(bd1, f32)[None, :]], 0).astype(bf),
        "wd2": cat([np.asarray(Wd2, f32), np.asarray(bd2, f32)[None, :]], 0).astype(bf),
        "ws1": np.asarray(Ws, f32)[:128].astype(bf),
        "ws2": np.asarray(Ws, f32)[128:200].astype(bf),
        "ws3": cat([np.asarray(Ws, f32)[200:], np.asarray(bs, f32)[None, :]], 0).astype(bf),
        "wa1": cat([np.asarray(Wa1, f32), np.asarray(ba1, f32)[None, :]], 0).astype(bf),
        "wa2": cat([np.asarray(Wa2, f32), np.asarray(ba2, f32)[None, :]], 0).astype(bf),
    }
    in_maps = []
    for c in range(NCORES):
        sl = slice(BL * c, BL * (c + 1))
        in_maps.append(dict(
            common,
            qi=np.ascontiguousarray(q_data[sl, :T].T).reshape(-1).astype(np.int32),
            qai=np.ascontiguousarray(qa_data[sl, :T].T).reshape(-1).astype(np.int32)))
    return in_maps


def kernel(**inputs):
    nc = _get_nc(T_FULL)
    in_maps = make_inmaps(**inputs)
    res = run_bass_kernel_spmd(nc, in_maps, core_ids=list(range(NCORES)), trace=False)
    return np.concatenate([res.results[c]["pred"] for c in range(NCORES)], axis=0)


# revision 15
# speedup vs baseline: 1.0650x; 1.0650x over previous
"""Bass/Trainium2 kernel for nn_DeepIRTModel (DKVMN knowledge tracing).

Strategy: data-parallel over batch (B=256 -> 32 per core on 8 cores).
Per core, sample index r = 32*t + b (t-major). Scan state X = Mv lives in
SBUF as (128 partitions = 32*g+b, free = (j,d)) with slot n = 13*g + j
(52 padded slots, pad weights = 0), d = DV = 200.

Per scan step t (engine assignment):
  DVE:  V = X * er_bcast ; V -= ad_bcast ; wG slices j>=KACT ; X -= wG ;
        W_sel = w_slab_bcast * I_sel_bcast
  ACT:  wG slices j < KACT ; readT PSUM->SBUF copies
  PE:   readT += X_dslice^T @ W_sel_j  (26 matmuls, PSUM accum over j)
Reads use X BEFORE the update (Tile's WAR deps order PE before the X write).
theta/beta/pred are batched matmul chains after the scan.
"""
import sys, types

sys.path.insert(0, '/opt/trn_rl_repo')
import numpy as np
import ml_dtypes


def _install_ntff_hook():
    try:
        import antenv
        if "antenv.axon_hooks" in sys.modules:
            return
        mod = types.ModuleType("antenv.axon_hooks")
        state = {"hook": None}
        mod.set_axon_ntff_profile_hook = lambda h: state.__setitem__("hook", h)
        mod.get_axon_ntff_profile_hook = lambda: state["hook"]
        sys.modules["antenv.axon_hooks"] = mod
        antenv.axon_hooks = mod
        from trn_agent_boot.trn_boot import _ntff_profile_via_ctypes
        mod.set_axon_ntff_profile_hook(_ntff_profile_via_ctypes('/opt/axon/libaxon_pjrt.so'))
    except Exception:
        pass


_install_ntff_hook()

import concourse.bass as bass
import concourse.bacc as bacc
import concourse.mybir as mybir
from concourse.tile import TileContext, add_dep_helper
from concourse.bass_utils import run_bass_kernel_spmd

BF16 = mybir.dt.bfloat16
F32 = mybir.dt.float32
I32 = mybir.dt.int32
AF = mybir.ActivationFunctionType
OP = mybir.AluOpType
AX = mybir.AxisListType

NQ, M, DK, DV, DS = 50000, 50, 50, 200, 50
B, T_FULL = 256, 200
SCALE = 3.0
NCORES = 8
BL = B // NCORES          # 32 batch rows per core
NJ = 13                   # slot groups per partition-subindex g in [0,4)
MP = 4 * NJ               # 52 padded slots
FREE = NJ * DV            # 2600 state free dim
KACT = 7                  # wG slices computed on ACT (j < KACT)


def build_nc(T=T_FULL, Tc=25, sim_safe=False):
    R = BL * T            # samples per core
    NCH = R // 128        # gather chunks of 128 rows
    assert R % 128 == 0
    NSEG = (R + 511) // 512
    nchunks = (T + Tc - 1) // Tc

    nc = bacc.Bacc(trn_type="TRN2")
    # ---- DRAM I/O ----
    qi_d = nc.dram_tensor("qi", [R], I32, kind="ExternalInput")
    qai_d = nc.dram_tensor("qai", [R], I32, kind="ExternalInput")
    qtab_d = nc.dram_tensor("qtab", [NQ + 1, DK], F32, kind="ExternalInput")
    qatab_d = nc.dram_tensor("qatab", [2 * NQ + 1, DV], F32, kind="ExternalInput")
    x0_d = nc.dram_tensor("x0", [128, FREE], BF16, kind="ExternalInput")
    isel_d = nc.dram_tensor("isel", [128, BL], BF16, kind="ExternalInput")
    ident_d = nc.dram_tensor("ident", [128, 128], F32, kind="ExternalInput")
    mkt_d = nc.dram_tensor("mkt", [DK, M], BF16, kind="ExternalInput")
    we1_d = nc.dram_tensor("we1", [128, DV], BF16, kind="ExternalInput")
    we2_d = nc.dram_tensor("we2", [73, DV], BF16, kind="ExternalInput")
    wad1_d = nc.dram_tensor("wad1", [128, DV], BF16, kind="ExternalInput")
    wad2_d = nc.dram_tensor("wad2", [73, DV], BF16, kind="ExternalInput")
    wd1_d = nc.dram_tensor("wd1", [DK + 1, DS], BF16, kind="ExternalInput")
    wd2_d = nc.dram_tensor("wd2", [DS + 1, 1], BF16, kind="ExternalInput")
    ws1_d = nc.dram_tensor("ws1", [128, DS], BF16, kind="ExternalInput")
    ws2_d = nc.dram_tensor("ws2", [72, DS], BF16, kind="ExternalInput")
    ws3_d = nc.dram_tensor("ws3", [DK + 1, DS], BF16, kind="ExternalInput")
    wa1_d = nc.dram_tensor("wa1", [DS + 1, DS], BF16, kind="ExternalInput")
    wa2_d = nc.dram_tensor("wa2", [DS + 1, 1], BF16, kind="ExternalInput")
    ones_d = nc.dram_tensor("ones", [1, BL * T], BF16, kind="ExternalInput")
    pred_d = nc.dram_tensor("pred", [BL, T], F32, kind="ExternalOutput")
    # internal DRAM roundtrip buffers
    er_dram = nc.dram_tensor("er_dram", [R, DV], BF16)
    ad_dram = nc.dram_tensor("ad_dram", [R, DV], BF16)
    w_dram = nc.dram_tensor("w_dram", [R, MP], F32)

    with TileContext(nc) as tc:
        with tc.tile_pool(name="pers", bufs=1) as pers, \
             tc.tile_pool(name="work", bufs=2) as work:

            # ---------- phase A: params + indices ----------
            ident = pers.tile([128, 128], F32)
            nc.sync.dma_start(out=ident, in_=ident_d[:, :])
            iselb = pers.tile([128, BL], BF16)
            nc.sync.dma_start(out=iselb, in_=isel_d[:, :])
            x = pers.tile([128, FREE], BF16)
            nc.sync.dma_start(out=x, in_=x0_d[:, :])
            mkt = pers.tile([DK, M], BF16)
            nc.sync.dma_start(out=mkt, in_=mkt_d[:, :])
            wd1 = pers.tile([DK + 1, DS], BF16)
            nc.sync.dma_start(out=wd1, in_=wd1_d[:, :])
            wd2 = pers.tile([DS + 1, 1], BF16)
            nc.sync.dma_start(out=wd2, in_=wd2_d[:, :])
            ws1 = pers.tile([128, DS], BF16)
            nc.sync.dma_start(out=ws1, in_=ws1_d[:, :])
            ws2 = pers.tile([72, DS], BF16)
            nc.sync.dma_start(out=ws2, in_=ws2_d[:, :])
            ws3 = pers.tile([DK + 1, DS], BF16)
            nc.sync.dma_start(out=ws3, in_=ws3_d[:, :])
            wa1 = pers.tile([DS + 1, DS], BF16)
            nc.sync.dma_start(out=wa1, in_=wa1_d[:, :])
            wa2 = pers.tile([DS + 1, 1], BF16)
            nc.sync.dma_start(out=wa2, in_=wa2_d[:, :])

            qi_sb = pers.tile([128, NCH], I32)
            nc.sync.dma_start(out=qi_sb, in_=qi_d.rearrange("(k p) -> p k", p=128))
            qai_sb = pers.tile([128, NCH], I32)
            nc.sync.dma_start(out=qai_sb, in_=qai_d.rearrange("(k p) -> p k", p=128))

            # persistent across phases
            qeT = pers.tile([DK + 1, R], BF16)
            nc.sync.dma_start(out=qeT[DK:DK + 1, :], in_=ones_d[:, :R])
            beta_sb = pers.tile([128, NCH], F32)
            w_scan = pers.tile([128, T * NJ], F32)
            w_scanb = pers.tile([128, T * NJ], BF16)
            readT_lo = pers.tile([128, R], BF16)
            readT_hi = pers.tile([72, R], BF16)
            zpad = pers.tile([128, 2], F32)
            nc.vector.memset(zpad, 0.0)

            # ---------- phases A2-C: gather, transpose, w/er/ad/beta ----------
            with tc.tile_pool(name="ac_sb", bufs=1) as ac_sb, \
                 tc.tile_pool(name="ac_ps", bufs=2, space="PSUM") as ac_ps:
                we1 = ac_sb.tile([128, DV], BF16)
                nc.sync.dma_start(out=we1, in_=we1_d[:, :])
                we2 = ac_sb.tile([73, DV], BF16)
                nc.sync.dma_start(out=we2, in_=we2_d[:, :])
                wad1 = ac_sb.tile([128, DV], BF16)
                nc.sync.dma_start(out=wad1, in_=wad1_d[:, :])
                wad2 = ac_sb.tile([73, DV], BF16)
                nc.sync.dma_start(out=wad2, in_=wad2_d[:, :])
                qaeT_lo = ac_sb.tile([128, R], BF16)
                qaeT_hi = ac_sb.tile([73, R], BF16)
                nc.sync.dma_start(out=qaeT_hi[72:73, :], in_=ones_d[:, :R])
                h2T = ac_sb.tile([DS + 1, R], BF16)
                nc.sync.dma_start(out=h2T[DS:DS + 1, :], in_=ones_d[:, :R])

                w_w_insts = []
                er_w_insts = {}
                ad_w_insts = {}
                for k in range(NCH):
                    qe_g = ac_sb.tile([128, DK], F32, tag="qe_g", bufs=3)
                    nc.gpsimd.indirect_dma_start(
                        out=qe_g, out_offset=None, in_=qtab_d[:, :],
                        in_offset=bass.IndirectOffsetOnAxis(ap=qi_sb[:, k:k + 1], axis=0))
                    qae_g = ac_sb.tile([128, DV], F32, tag="qae_g", bufs=3)
                    nc.gpsimd.indirect_dma_start(
                        out=qae_g, out_offset=None, in_=qatab_d[:, :],
                        in_offset=bass.IndirectOffsetOnAxis(ap=qai_sb[:, k:k + 1], axis=0))
                    pt = ac_ps.tile([128, 128], F32, tag="pt", bufs=2)
                    nc.tensor.transpose(out=pt[:DK, :], in_=qe_g, identity=ident)
                    nc.scalar.copy(qeT[:DK, 128 * k:128 * (k + 1)], pt[:DK, :])
                    pt2 = ac_ps.tile([128, 128], F32, tag="pt", bufs=2)
                    nc.tensor.transpose(out=pt2, in_=qae_g[:, :128], identity=ident)
                    nc.scalar.copy(qaeT_lo[:, 128 * k:128 * (k + 1)], pt2)
                    pt3 = ac_ps.tile([128, 128], F32, tag="pt", bufs=2)
                    nc.tensor.transpose(out=pt3[:72, :], in_=qae_g[:, 128:200], identity=ident)
                    nc.scalar.copy(qaeT_hi[:72, 128 * k:128 * (k + 1)], pt3[:72, :])

                for k in range(NCH):
                    lg = ac_ps.tile([128, M], F32, tag="lg", bufs=2)
                    nc.tensor.matmul(lg, lhsT=qeT[:DK, 128 * k:128 * (k + 1)], rhs=mkt,
                                     start=True, stop=True)
                    ex = ac_sb.tile([128, M], F32, tag="ex", bufs=3)
                    nc.scalar.activation(out=ex, in_=lg, func=AF.Exp)
                    sm = ac_sb.tile([128, 1], F32, tag="sm", bufs=3)
                    nc.vector.reduce_sum(sm, ex, axis=AX.X)
                    rc = ac_sb.tile([128, 1], F32, tag="rc", bufs=3)
                    nc.vector.reciprocal(rc, sm)
                    wn = ac_sb.tile([128, M], F32, tag="wn", bufs=3)
                    nc.vector.tensor_scalar(out=wn, in0=ex, scalar1=rc[:, 0:1],
                                            scalar2=None, op0=OP.mult)
                    w_w_insts.append(nc.sync.dma_start(
                        out=w_dram[128 * k:128 * (k + 1), :M], in_=wn))
                    w_w_insts.append(nc.sync.dma_start(
                        out=w_dram[128 * k:128 * (k + 1), M:MP], in_=zpad[:, :2]))

                for k in range(NCH):
                    ep = ac_ps.tile([128, DV], F32, tag="eap", bufs=2, name="ep")
                    nc.tensor.matmul(ep, lhsT=qaeT_lo[:, 128 * k:128 * (k + 1)], rhs=we1,
                                     start=True, stop=False)
                    nc.tensor.matmul(ep, lhsT=qaeT_hi[:, 128 * k:128 * (k + 1)], rhs=we2,
                                     start=False, stop=True)
                    ero = ac_sb.tile([128, DV], BF16, tag="ero", bufs=3)
                    nc.scalar.activation(out=ero, in_=ep, func=AF.Sigmoid)
                    er_w_insts[k] = nc.sync.dma_start(
                        out=er_dram[128 * k:128 * (k + 1), :], in_=ero)

                for k in range(NCH):
                    ap_ = ac_ps.tile([128, DV], F32, tag="eap", bufs=2, name="ap_")
                    nc.tensor.matmul(ap_, lhsT=qaeT_lo[:, 128 * k:128 * (k + 1)], rhs=wad1,
                                     start=True, stop=False)
                    nc.tensor.matmul(ap_, lhsT=qaeT_hi[:, 128 * k:128 * (k + 1)], rhs=wad2,
                                     start=False, stop=True)
                    ado = ac_sb.tile([128, DV], BF16, tag="ado", bufs=3)
                    nc.scalar.activation(out=ado, in_=ap_, func=AF.Tanh)
                    ad_w_insts[k] = nc.sync.dma_start(
                        out=ad_dram[128 * k:128 * (k + 1), :], in_=ado)

                # beta chain
                for s in range(NSEG):
                    n0, n1 = 512 * s, min(512 * (s + 1), R)
                    hp = ac_ps.tile([DS, 512], F32, tag="hp", bufs=1)
                    nc.tensor.matmul(hp[:, :n1 - n0], lhsT=wd1, rhs=qeT[:, n0:n1],
                                     start=True, stop=True)
                    nc.scalar.activation(out=h2T[:DS, n0:n1], in_=hp[:, :n1 - n0],
                                         func=AF.Tanh)
                beta_ps = ac_ps.tile([128, NCH], F32, tag="beta_ps", bufs=1)
                for k in range(NCH):
                    nc.tensor.matmul(beta_ps[:, k:k + 1],
                                     lhsT=h2T[:, 128 * k:128 * (k + 1)],
                                     rhs=wd2, start=True, stop=True)
                nc.scalar.copy(beta_sb, beta_ps)

                # w_scan load: (128=(4b+g), (t,13)) from w_dram[(32t+b), 13g+j]
                if sim_safe:
                    nc.vector.memset(w_scan, 0.0)
                for g in range(4):
                    src = w_dram[:, NJ * g:NJ * (g + 1)].rearrange(
                        "(t b) j -> b t j", b=BL)
                    dst = w_scan[32 * g:32 * (g + 1), :].rearrange("b (t j) -> b t j", j=NJ)
                    ld = nc.sync.dma_start(out=dst, in_=src)
                    for wi in w_w_insts:
                        add_dep_helper(ld.ins, wi.ins, reason="w_dram roundtrip")
                nc.vector.tensor_copy(w_scanb, w_scan)

            # ---------- phase D: the scan ----------
            er_src = er_dram.rearrange("(t b) d -> t b d", b=BL)
            ad_src = ad_dram.rearrange("(t b) d -> t b d", b=BL)
            with tc.tile_pool(name="d_ps", bufs=2, space="PSUM") as d_ps:
                for ch in range(nchunks):
                    t0, t1 = Tc * ch, min(Tc * (ch + 1), T)
                    tl = t1 - t0
                    er_ch = work.tile([128, Tc * DV], BF16, tag="er_ch", bufs=2)
                    ad_ch = work.tile([128, Tc * DV], BF16, tag="ad_ch", bufs=2)
                    if sim_safe:
                        nc.vector.memset(er_ch, 0.0)
                        nc.vector.memset(ad_ch, 0.0)
                    ks = range(t0 // 4, (t1 + 3) // 4)
                    for g in range(4):
                        ldi = nc.sync.dma_start(
                            out=er_ch[32 * g:32 * (g + 1), :tl * DV].rearrange(
                                "b (t d) -> b t d", d=DV),
                            in_=er_src[t0:t1, :, :].rearrange("t b d -> b t d"))
                        for k in ks:
                            add_dep_helper(ldi.ins, er_w_insts[k].ins, reason="er roundtrip")
                        lda = nc.sync.dma_start(
                            out=ad_ch[32 * g:32 * (g + 1), :tl * DV].rearrange(
                                "b (t d) -> b t d", d=DV),
                            in_=ad_src[t0:t1, :, :].rearrange("t b d -> b t d"))
                        for k in ks:
                            add_dep_helper(lda.ins, ad_w_insts[k].ins, reason="ad roundtrip")

                    for tt in range(tl):
                        t = t0 + tt
                        er_t = er_ch[:, tt * DV:(tt + 1) * DV]
                        ad_t = ad_ch[:, tt * DV:(tt + 1) * DV]
                        er_b = er_t.rearrange("p (o d) -> p o d", o=1).to_broadcast(
                            [128, NJ, DV])
                        ad_b = ad_t.rearrange("p (o d) -> p o d", o=1).to_broadcast(
                            [128, NJ, DV])

                        wsel = work.tile([128, NJ * BL], BF16, tag="wsel", bufs=2)
                        nc.gpsimd.tensor_tensor(
                            out=wsel.rearrange("p (j b) -> p j b", j=NJ),
                            in0=w_scanb[:, t * NJ:(t + 1) * NJ].rearrange(
                                "p (j o) -> p j o", o=1).to_broadcast([128, NJ, BL]),
                            in1=iselb.rearrange("p (o b) -> p o b", o=1).to_broadcast(
                                [128, NJ, BL]),
                            op=OP.mult)

                        # PE: readT (uses X before this step's update)
                        rlo = d_ps.tile([128, BL], F32, tag="rlo", bufs=2)
                        rhi = d_ps.tile([72, BL], F32, tag="rhi", bufs=2)
                        for j in range(NJ):
                            nc.tensor.matmul(rlo, lhsT=x[:, j * DV:j * DV + 128],
                                             rhs=wsel[:, j * BL:(j + 1) * BL],
                                             start=(j == 0), stop=(j == NJ - 1))
                        for j in range(NJ):
                            nc.tensor.matmul(rhi, lhsT=x[:, j * DV + 128:(j + 1) * DV],
                                             rhs=wsel[:, j * BL:(j + 1) * BL],
                                             start=(j == 0), stop=(j == NJ - 1))
                        nc.scalar.copy(readT_lo[:, BL * t:BL * (t + 1)], rlo)
                        nc.scalar.copy(readT_hi[:, BL * t:BL * (t + 1)], rhi[:72, :])

                        # DVE: V = X*er_b ; V -= ad_b (split so ACT starts early)
                        v = work.tile([128, FREE], BF16, tag="v", bufs=2)
                        v3 = v.rearrange("p (j d) -> p j d", j=NJ)
                        x3 = x.rearrange("p (j d) -> p j d", j=NJ)
                        ka = KACT
                        nc.vector.tensor_tensor(out=v3[:, :ka, :], in0=x3[:, :ka, :],
                                                in1=er_b[:, :ka, :], op=OP.mult)
                        nc.vector.tensor_tensor(out=v3[:, :ka, :], in0=v3[:, :ka, :],
                                                in1=ad_b[:, :ka, :], op=OP.subtract)
                        nc.vector.tensor_tensor(out=v3[:, ka:, :], in0=x3[:, ka:, :],
                                                in1=er_b[:, ka:, :], op=OP.mult)
                        nc.vector.tensor_tensor(out=v3[:, ka:, :], in0=v3[:, ka:, :],
                                                in1=ad_b[:, ka:, :], op=OP.subtract)
                        wg = work.tile([128, FREE], BF16, tag="wg", bufs=2)
                        for j in range(KACT):
                            wcol = w_scan[:, t * NJ + j:t * NJ + j + 1]
                            nc.scalar.activation(out=wg[:, j * DV:(j + 1) * DV],
                                                 in_=v[:, j * DV:(j + 1) * DV],
                                                 func=AF.Copy, scale=wcol)
                        for j in range(KACT, NJ):
                            wcol = w_scan[:, t * NJ + j:t * NJ + j + 1]
                            nc.vector.tensor_scalar(out=wg[:, j * DV:(j + 1) * DV],
                                                    in0=v[:, j * DV:(j + 1) * DV],
                                                    scalar1=wcol, scalar2=None,
                                                    op0=OP.mult)
                        nc.vector.tensor_tensor(out=x, in0=x, in1=wg, op=OP.subtract)

            # ---------- phase E: summ/theta/pred ----------
            with tc.tile_pool(name="e_ps", bufs=2, space="PSUM") as e_ps:
                summT = pers.tile([DS + 1, R], BF16)
                nc.sync.dma_start(out=summT[DS:DS + 1, :], in_=ones_d[:, :R])
                for s in range(NSEG):
                    n0, n1 = 512 * s, min(512 * (s + 1), R)
                    sp = e_ps.tile([DS, 512], F32, tag="sp", bufs=2)
                    nc.tensor.matmul(sp[:, :n1 - n0], lhsT=ws1, rhs=readT_lo[:, n0:n1],
                                     start=True, stop=False)
                    nc.tensor.matmul(sp[:, :n1 - n0], lhsT=ws2, rhs=readT_hi[:72, n0:n1],
                                     start=False, stop=False)
                    nc.tensor.matmul(sp[:, :n1 - n0], lhsT=ws3, rhs=qeT[:, n0:n1],
                                     start=False, stop=True)
                    nc.scalar.activation(out=summT[:DS, n0:n1], in_=sp[:, :n1 - n0],
                                         func=AF.Tanh)
                hT = pers.tile([DS + 1, R], BF16)
                nc.sync.dma_start(out=hT[DS:DS + 1, :], in_=ones_d[:, :R])
                for s in range(NSEG):
                    n0, n1 = 512 * s, min(512 * (s + 1), R)
                    hp2 = e_ps.tile([DS, 512], F32, tag="hp2", bufs=2)
                    nc.tensor.matmul(hp2[:, :n1 - n0], lhsT=wa1, rhs=summT[:, n0:n1],
                                     start=True, stop=True)
                    nc.scalar.activation(out=hT[:DS, n0:n1], in_=hp2[:, :n1 - n0],
                                         func=AF.Tanh)
                th_ps = e_ps.tile([128, NCH], F32, tag="th_ps", bufs=1)
                for k in range(NCH):
                    nc.tensor.matmul(th_ps[:, k:k + 1], lhsT=hT[:, 128 * k:128 * (k + 1)],
                                     rhs=wa2, start=True, stop=True)
                pre = pers.tile([128, NCH], F32)
                nc.vector.scalar_tensor_tensor(out=pre, in0=th_ps, scalar=SCALE,
                                               in1=beta_sb, op0=OP.mult, op1=OP.subtract)
                pred_sb = pers.tile([128, NCH], F32)
                nc.scalar.activation(out=pred_sb, in_=pre, func=AF.Sigmoid)
                # pred_sb[p=32u+i, k] -> pred_d[b=i, t=4k+u]
                nc.sync.dma_start(
                    out=pred_d.rearrange("b (k u) -> u b k", u=4),
                    in_=pred_sb[:, :])

    return nc


_NC_CACHE = {}


def _get_nc(T=T_FULL):
    if T not in _NC_CACHE:
        n = build_nc(T=T)
        n.compile()
        _NC_CACHE[T] = n
    return _NC_CACHE[T]


def make_inmaps(q_data, qa_data, q_tab, qa_tab, Mk, Mv0, Ws, bs, Wa1, ba1, Wa2,
                ba2, Wd1, bd1, Wd2, bd2, We, be, Wad, bad, T=T_FULL):
    bf = ml_dtypes.bfloat16
    f32 = np.float32
    q_data = np.asarray(q_data)
    qa_data = np.asarray(qa_data)

    mv0p = np.zeros((MP, DV), f32)
    mv0p[:M] = np.asarray(Mv0, f32)
    x0 = mv0p.reshape(4, NJ, DV)[:, None].repeat(BL, 1).reshape(128, FREE).astype(bf)
    isel = np.tile(np.eye(BL, dtype=f32), (4, 1)).astype(bf)  # (128, 32), g-major
    ident = np.eye(128, dtype=f32)

    cat = np.concatenate
    common = {
        "qtab": np.ascontiguousarray(np.asarray(q_tab, f32)),
        "qatab": np.ascontiguousarray(np.asarray(qa_tab, f32)),
        "x0": x0, "isel": isel, "ident": ident,
        "ones": np.ones((1, BL * T), f32).astype(bf),
        "mkt": np.asarray(Mk, f32).T.copy().astype(bf),
        "we1": np.asarray(We, f32)[:128].astype(bf),
        "we2": cat([np.asarray(We, f32)[128:], np.asarray(be, f32)[None, :]], 0).astype(bf),
        "wad1": np.asarray(Wad, f32)[:128].astype(bf),
        "wad2": cat([np.asarray(Wad, f32)[128:], np.asarray(bad, f32)[None, :]], 0).astype(bf),
        "wd1": cat([np.asarray(Wd1, f32), np.asarray(bd1, f32)[None, :]], 0).astype(bf),
        "wd2": cat([np.asarray(Wd2, f32), np.asarray(bd2, f32)[None, :]], 0).astype(bf),
        "ws1": np.asarray(Ws, f32)[:128].astype(bf),
        "ws2": np.asarray(Ws, f32)[128:200].astype(bf),
        "ws3": cat([np.asarray(Ws, f32)[200:], np.asarray(bs, f32)[None, :]], 0).astype(bf),
        "wa1": cat([np.asarray(Wa1, f32), np.asarray(ba1, f32)[None, :]], 0).astype(bf),
        "wa2": cat([np.asarray(Wa2, f32), np.asarray(ba2, f32)[None, :]], 0).astype(bf),
    }
    in_maps = []
    for c in range(NCORES):
        sl = slice(BL * c, BL * (c + 1))
        in_maps.append(dict(
            common,
            qi=np.ascontiguousarray(q_data[sl, :T].T).reshape(-1).astype(np.int32),
            qai=np.ascontiguousarray(qa_data[sl, :T].T).reshape(-1).astype(np.int32)))
    return in_maps


def kernel(**inputs):
    nc = _get_nc(T_FULL)
    in_maps = make_inmaps(**inputs)
    res = run_bass_kernel_spmd(nc, in_maps, core_ids=list(range(NCORES)), trace=False)
    return np.concatenate([res.results[c]["pred"] for c in range(NCORES)], axis=0)


# revision 16
# speedup vs baseline: 1.0959x; 1.0290x over previous
"""Bass/Trainium2 kernel for nn_DeepIRTModel (DKVMN knowledge tracing).

Strategy: data-parallel over batch (B=256 -> 32 per core on 8 cores).
Per core, sample index r = 32*t + b (t-major). Scan state X = Mv lives in
SBUF as (128 partitions = 32*g+b, free = (j,d)) with slot n = 13*g + j
(52 padded slots, pad weights = 0), d = DV = 200.

Per scan step t (engine assignment):
  DVE:  V = X * er_bcast ; V -= ad_bcast ; wG slices j>=KACT ; X -= wG ;
        W_sel = w_slab_bcast * I_sel_bcast
  ACT:  wG slices j < KACT ; readT PSUM->SBUF copies
  PE:   readT += X_dslice^T @ W_sel_j  (26 matmuls, PSUM accum over j)
Reads use X BEFORE the update (Tile's WAR deps order PE before the X write).
theta/beta/pred are batched matmul chains after the scan.
"""
import sys, types

sys.path.insert(0, '/opt/trn_rl_repo')
import numpy as np
import ml_dtypes


def _install_ntff_hook():
    try:
        import antenv
        if "antenv.axon_hooks" in sys.modules:
            return
        mod = types.ModuleType("antenv.axon_hooks")
        state = {"hook": None}
        mod.set_axon_ntff_profile_hook = lambda h: state.__setitem__("hook", h)
        mod.get_axon_ntff_profile_hook = lambda: state["hook"]
        sys.modules["antenv.axon_hooks"] = mod
        antenv.axon_hooks = mod
        from trn_agent_boot.trn_boot import _ntff_profile_via_ctypes
        mod.set_axon_ntff_profile_hook(_ntff_profile_via_ctypes('/opt/axon/libaxon_pjrt.so'))
    except Exception:
        pass


_install_ntff_hook()

import concourse.bass as bass
import concourse.bacc as bacc
import concourse.mybir as mybir
from concourse.tile import TileContext, add_dep_helper
from concourse.bass_utils import run_bass_kernel_spmd

BF16 = mybir.dt.bfloat16
F32 = mybir.dt.float32
I32 = mybir.dt.int32
AF = mybir.ActivationFunctionType
OP = mybir.AluOpType
AX = mybir.AxisListType

NQ, M, DK, DV, DS = 50000, 50, 50, 200, 50
B, T_FULL = 256, 200
SCALE = 3.0
NCORES = 8
BL = B // NCORES          # 32 batch rows per core
NJ = 13                   # slot groups per partition-subindex g in [0,4)
MP = 4 * NJ               # 52 padded slots
FREE = NJ * DV            # 2600 state free dim
KACT = 6                  # wG slices computed on ACT (j < KACT)


def build_nc(T=T_FULL, Tc=25, sim_safe=False):
    R = BL * T            # samples per core
    NCH = R // 128        # gather chunks of 128 rows
    assert R % 128 == 0
    NSEG = (R + 511) // 512
    nchunks = (T + Tc - 1) // Tc

    nc = bacc.Bacc(trn_type="TRN2")
    # ---- DRAM I/O ----
    qi_d = nc.dram_tensor("qi", [R], I32, kind="ExternalInput")
    qai_d = nc.dram_tensor("qai", [R], I32, kind="ExternalInput")
    qtab_d = nc.dram_tensor("qtab", [NQ + 1, DK], F32, kind="ExternalInput")
    qatab_d = nc.dram_tensor("qatab", [2 * NQ + 1, DV], F32, kind="ExternalInput")
    x0_d = nc.dram_tensor("x0", [128, FREE], BF16, kind="ExternalInput")
    isel_d = nc.dram_tensor("isel", [128, BL], BF16, kind="ExternalInput")
    ident_d = nc.dram_tensor("ident", [128, 128], F32, kind="ExternalInput")
    mkt_d = nc.dram_tensor("mkt", [DK, M], BF16, kind="ExternalInput")
    we1_d = nc.dram_tensor("we1", [128, DV], BF16, kind="ExternalInput")
    we2_d = nc.dram_tensor("we2", [73, DV], BF16, kind="ExternalInput")
    wad1_d = nc.dram_tensor("wad1", [128, DV], BF16, kind="ExternalInput")
    wad2_d = nc.dram_tensor("wad2", [73, DV], BF16, kind="ExternalInput")
    wd1_d = nc.dram_tensor("wd1", [DK + 1, DS], BF16, kind="ExternalInput")
    wd2_d = nc.dram_tensor("wd2", [DS + 1, 1], BF16, kind="ExternalInput")
    ws1_d = nc.dram_tensor("ws1", [128, DS], BF16, kind="ExternalInput")
    ws2_d = nc.dram_tensor("ws2", [72, DS], BF16, kind="ExternalInput")
    ws3_d = nc.dram_tensor("ws3", [DK + 1, DS], BF16, kind="ExternalInput")
    wa1_d = nc.dram_tensor("wa1", [DS + 1, DS], BF16, kind="ExternalInput")
    wa2_d = nc.dram_tensor("wa2", [DS + 1, 1], BF16, kind="ExternalInput")
    ones_d = nc.dram_tensor("ones", [1, BL * T], BF16, kind="ExternalInput")
    pred_d = nc.dram_tensor("pred", [BL, T], F32, kind="ExternalOutput")
    # internal DRAM roundtrip buffers
    er_dram = nc.dram_tensor("er_dram", [R, DV], BF16)
    ad_dram = nc.dram_tensor("ad_dram", [R, DV], BF16)
    w_dram = nc.dram_tensor("w_dram", [R, MP], F32)

    with TileContext(nc) as tc:
        with tc.tile_pool(name="pers", bufs=1) as pers, \
             tc.tile_pool(name="work", bufs=2) as work:

            # ---------- phase A: params + indices ----------
            ident = pers.tile([128, 128], F32)
            nc.sync.dma_start(out=ident, in_=ident_d[:, :])
            iselb = pers.tile([128, BL], BF16)
            nc.sync.dma_start(out=iselb, in_=isel_d[:, :])
            x = pers.tile([128, FREE], BF16)
            nc.sync.dma_start(out=x, in_=x0_d[:, :])
            mkt = pers.tile([DK, M], BF16)
            nc.sync.dma_start(out=mkt, in_=mkt_d[:, :])
            wd1 = pers.tile([DK + 1, DS], BF16)
            nc.sync.dma_start(out=wd1, in_=wd1_d[:, :])
            wd2 = pers.tile([DS + 1, 1], BF16)
            nc.sync.dma_start(out=wd2, in_=wd2_d[:, :])
            ws1 = pers.tile([128, DS], BF16)
            nc.sync.dma_start(out=ws1, in_=ws1_d[:, :])
            ws2 = pers.tile([72, DS], BF16)
            nc.sync.dma_start(out=ws2, in_=ws2_d[:, :])
            ws3 = pers.tile([DK + 1, DS], BF16)
            nc.sync.dma_start(out=ws3, in_=ws3_d[:, :])
            wa1 = pers.tile([DS + 1, DS], BF16)
            nc.sync.dma_start(out=wa1, in_=wa1_d[:, :])
            wa2 = pers.tile([DS + 1, 1], BF16)
            nc.sync.dma_start(out=wa2, in_=wa2_d[:, :])

            qi_sb = pers.tile([128, NCH], I32)
            nc.sync.dma_start(out=qi_sb, in_=qi_d.rearrange("(k p) -> p k", p=128))
            qai_sb = pers.tile([128, NCH], I32)
            nc.sync.dma_start(out=qai_sb, in_=qai_d.rearrange("(k p) -> p k", p=128))

            # persistent across phases
            qeT = pers.tile([DK + 1, R], BF16)
            nc.sync.dma_start(out=qeT[DK:DK + 1, :], in_=ones_d[:, :R])
            beta_sb = pers.tile([128, NCH], F32)
            w_scan = pers.tile([128, T * NJ], F32)
            w_scanb = pers.tile([128, T * NJ], BF16)
            readT_lo = pers.tile([128, R], BF16)
            readT_hi = pers.tile([72, R], BF16)
            zpad = pers.tile([128, 2], F32)
            nc.vector.memset(zpad, 0.0)

            # ---------- phases A2-C: gather, transpose, w/er/ad/beta ----------
            with tc.tile_pool(name="ac_sb", bufs=1) as ac_sb, \
                 tc.tile_pool(name="ac_ps", bufs=2, space="PSUM") as ac_ps:
                we1 = ac_sb.tile([128, DV], BF16)
                nc.sync.dma_start(out=we1, in_=we1_d[:, :])
                we2 = ac_sb.tile([73, DV], BF16)
                nc.sync.dma_start(out=we2, in_=we2_d[:, :])
                wad1 = ac_sb.tile([128, DV], BF16)
                nc.sync.dma_start(out=wad1, in_=wad1_d[:, :])
                wad2 = ac_sb.tile([73, DV], BF16)
                nc.sync.dma_start(out=wad2, in_=wad2_d[:, :])
                qaeT_lo = ac_sb.tile([128, R], BF16)
                qaeT_hi = ac_sb.tile([73, R], BF16)
                nc.sync.dma_start(out=qaeT_hi[72:73, :], in_=ones_d[:, :R])
                h2T = ac_sb.tile([DS + 1, R], BF16)
                nc.sync.dma_start(out=h2T[DS:DS + 1, :], in_=ones_d[:, :R])

                w_w_insts = []
                er_w_insts = {}
                ad_w_insts = {}
                for k in range(NCH):
                    qe_g = ac_sb.tile([128, DK], F32, tag="qe_g", bufs=3)
                    nc.gpsimd.indirect_dma_start(
                        out=qe_g, out_offset=None, in_=qtab_d[:, :],
                        in_offset=bass.IndirectOffsetOnAxis(ap=qi_sb[:, k:k + 1], axis=0))
                    qae_g = ac_sb.tile([128, DV], F32, tag="qae_g", bufs=3)
                    nc.gpsimd.indirect_dma_start(
                        out=qae_g, out_offset=None, in_=qatab_d[:, :],
                        in_offset=bass.IndirectOffsetOnAxis(ap=qai_sb[:, k:k + 1], axis=0))
                    pt = ac_ps.tile([128, 128], F32, tag="pt", bufs=2)
                    nc.tensor.transpose(out=pt[:DK, :], in_=qe_g, identity=ident)
                    nc.scalar.copy(qeT[:DK, 128 * k:128 * (k + 1)], pt[:DK, :])
                    pt2 = ac_ps.tile([128, 128], F32, tag="pt", bufs=2)
                    nc.tensor.transpose(out=pt2, in_=qae_g[:, :128], identity=ident)
                    nc.scalar.copy(qaeT_lo[:, 128 * k:128 * (k + 1)], pt2)
                    pt3 = ac_ps.tile([128, 128], F32, tag="pt", bufs=2)
                    nc.tensor.transpose(out=pt3[:72, :], in_=qae_g[:, 128:200], identity=ident)
                    nc.scalar.copy(qaeT_hi[:72, 128 * k:128 * (k + 1)], pt3[:72, :])

                for k in range(NCH):
                    lg = ac_ps.tile([128, M], F32, tag="lg", bufs=2)
                    nc.tensor.matmul(lg, lhsT=qeT[:DK, 128 * k:128 * (k + 1)], rhs=mkt,
                                     start=True, stop=True)
                    ex = ac_sb.tile([128, M], F32, tag="ex", bufs=3)
                    nc.scalar.activation(out=ex, in_=lg, func=AF.Exp)
                    sm = ac_sb.tile([128, 1], F32, tag="sm", bufs=3)
                    nc.vector.reduce_sum(sm, ex, axis=AX.X)
                    rc = ac_sb.tile([128, 1], F32, tag="rc", bufs=3)
                    nc.vector.reciprocal(rc, sm)
                    wn = ac_sb.tile([128, M], F32, tag="wn", bufs=3)
                    nc.vector.tensor_scalar(out=wn, in0=ex, scalar1=rc[:, 0:1],
                                            scalar2=None, op0=OP.mult)
                    w_w_insts.append(nc.sync.dma_start(
                        out=w_dram[128 * k:128 * (k + 1), :M], in_=wn))
                    w_w_insts.append(nc.sync.dma_start(
                        out=w_dram[128 * k:128 * (k + 1), M:MP], in_=zpad[:, :2]))

                for k in range(NCH):
                    ep = ac_ps.tile([128, DV], F32, tag="eap", bufs=2, name="ep")
                    nc.tensor.matmul(ep, lhsT=qaeT_lo[:, 128 * k:128 * (k + 1)], rhs=we1,
                                     start=True, stop=False)
                    nc.tensor.matmul(ep, lhsT=qaeT_hi[:, 128 * k:128 * (k + 1)], rhs=we2,
                                     start=False, stop=True)
                    ero = ac_sb.tile([128, DV], BF16, tag="ero", bufs=3)
                    nc.scalar.activation(out=ero, in_=ep, func=AF.Sigmoid)
                    er_w_insts[k] = nc.sync.dma_start(
                        out=er_dram[128 * k:128 * (k + 1), :], in_=ero)

                for k in range(NCH):
                    ap_ = ac_ps.tile([128, DV], F32, tag="eap", bufs=2, name="ap_")
                    nc.tensor.matmul(ap_, lhsT=qaeT_lo[:, 128 * k:128 * (k + 1)], rhs=wad1,
                                     start=True, stop=False)
                    nc.tensor.matmul(ap_, lhsT=qaeT_hi[:, 128 * k:128 * (k + 1)], rhs=wad2,
                                     start=False, stop=True)
                    ado = ac_sb.tile([128, DV], BF16, tag="ado", bufs=3)
                    nc.scalar.activation(out=ado, in_=ap_, func=AF.Tanh)
                    ad_w_insts[k] = nc.sync.dma_start(
                        out=ad_dram[128 * k:128 * (k + 1), :], in_=ado)

                # beta chain
                for s in range(NSEG):
                    n0, n1 = 512 * s, min(512 * (s + 1), R)
                    hp = ac_ps.tile([DS, 512], F32, tag="hp", bufs=1)
                    nc.tensor.matmul(hp[:, :n1 - n0], lhsT=wd1, rhs=qeT[:, n0:n1],
                                     start=True, stop=True)
                    nc.scalar.activation(out=h2T[:DS, n0:n1], in_=hp[:, :n1 - n0],
                                         func=AF.Tanh)
                beta_ps = ac_ps.tile([128, NCH], F32, tag="beta_ps", bufs=1)
                for k in range(NCH):
                    nc.tensor.matmul(beta_ps[:, k:k + 1],
                                     lhsT=h2T[:, 128 * k:128 * (k + 1)],
                                     rhs=wd2, start=True, stop=True)
                nc.scalar.copy(beta_sb, beta_ps)

                # w_scan load: (128=(4b+g), (t,13)) from w_dram[(32t+b), 13g+j]
                if sim_safe:
                    nc.vector.memset(w_scan, 0.0)
                for g in range(4):
                    src = w_dram[:, NJ * g:NJ * (g + 1)].rearrange(
                        "(t b) j -> b t j", b=BL)
                    dst = w_scan[32 * g:32 * (g + 1), :].rearrange("b (t j) -> b t j", j=NJ)
                    ld = nc.sync.dma_start(out=dst, in_=src)
                    for wi in w_w_insts:
                        add_dep_helper(ld.ins, wi.ins, reason="w_dram roundtrip")
                nc.vector.tensor_copy(w_scanb, w_scan)

            # ---------- phase D: the scan ----------
            er_src = er_dram.rearrange("(t b) d -> t b d", b=BL)
            ad_src = ad_dram.rearrange("(t b) d -> t b d", b=BL)
            with tc.tile_pool(name="d_ps", bufs=2, space="PSUM") as d_ps:
                for ch in range(nchunks):
                    t0, t1 = Tc * ch, min(Tc * (ch + 1), T)
                    tl = t1 - t0
                    er_ch = work.tile([128, Tc * DV], BF16, tag="er_ch", bufs=2)
                    ad_ch = work.tile([128, Tc * DV], BF16, tag="ad_ch", bufs=2)
                    if sim_safe:
                        nc.vector.memset(er_ch, 0.0)
                        nc.vector.memset(ad_ch, 0.0)
                    ks = range(t0 // 4, (t1 + 3) // 4)
                    for g in range(4):
                        ldi = nc.sync.dma_start(
                            out=er_ch[32 * g:32 * (g + 1), :tl * DV].rearrange(
                                "b (t d) -> b t d", d=DV),
                            in_=er_src[t0:t1, :, :].rearrange("t b d -> b t d"))
                        for k in ks:
                            add_dep_helper(ldi.ins, er_w_insts[k].ins, reason="er roundtrip")
                        lda = nc.gpsimd.dma_start(
                            out=ad_ch[32 * g:32 * (g + 1), :tl * DV].rearrange(
                                "b (t d) -> b t d", d=DV),
                            in_=ad_src[t0:t1, :, :].rearrange("t b d -> b t d"))
                        for k in ks:
                            add_dep_helper(lda.ins, ad_w_insts[k].ins, reason="ad roundtrip")

                    for tt in range(tl):
                        t = t0 + tt
                        er_t = er_ch[:, tt * DV:(tt + 1) * DV]
                        ad_t = ad_ch[:, tt * DV:(tt + 1) * DV]
                        er_b = er_t.rearrange("p (o d) -> p o d", o=1).to_broadcast(
                            [128, NJ, DV])
                        ad_b = ad_t.rearrange("p (o d) -> p o d", o=1).to_broadcast(
                            [128, NJ, DV])

                        wsel = work.tile([128, NJ * BL], BF16, tag="wsel", bufs=2)
                        nc.gpsimd.tensor_tensor(
                            out=wsel.rearrange("p (j b) -> p j b", j=NJ),
                            in0=w_scanb[:, t * NJ:(t + 1) * NJ].rearrange(
                                "p (j o) -> p j o", o=1).to_broadcast([128, NJ, BL]),
                            in1=iselb.rearrange("p (o b) -> p o b", o=1).to_broadcast(
                                [128, NJ, BL]),
                            op=OP.mult)

                        # PE: readT (uses X before this step's update)
                        rlo = d_ps.tile([128, BL], F32, tag="rlo", bufs=2)
                        rhi = d_ps.tile([72, BL], F32, tag="rhi", bufs=2)
                        for j in range(NJ):
                            nc.tensor.matmul(rlo, lhsT=x[:, j * DV:j * DV + 128],
                                             rhs=wsel[:, j * BL:(j + 1) * BL],
                                             start=(j == 0), stop=(j == NJ - 1))
                        for j in range(NJ):
                            nc.tensor.matmul(rhi, lhsT=x[:, j * DV + 128:(j + 1) * DV],
                                             rhs=wsel[:, j * BL:(j + 1) * BL],
                                             start=(j == 0), stop=(j == NJ - 1))
                        nc.scalar.copy(readT_lo[:, BL * t:BL * (t + 1)], rlo)
                        nc.scalar.copy(readT_hi[:, BL * t:BL * (t + 1)], rhi[:72, :])

                        # DVE: V = X*er_b ; V -= ad_b (split so ACT starts early)
                        ka = KACT
                        kb = NJ - KACT
                        va = work.tile([128, ka * DV], BF16, tag="va", bufs=2)
                        vb = work.tile([128, kb * DV], BF16, tag="vb", bufs=2)
                        va3 = va.rearrange("p (j d) -> p j d", j=ka)
                        vb3 = vb.rearrange("p (j d) -> p j d", j=kb)
                        x3 = x.rearrange("p (j d) -> p j d", j=NJ)
                        i1 = nc.vector.tensor_tensor(out=va3, in0=x3[:, :ka, :],
                                                     in1=er_b[:, :ka, :], op=OP.mult)
                        i2 = nc.vector.tensor_tensor(out=va3, in0=va3,
                                                     in1=ad_b[:, :ka, :], op=OP.subtract)
                        i3 = nc.vector.tensor_tensor(out=vb3, in0=x3[:, ka:, :],
                                                     in1=er_b[:, ka:, :], op=OP.mult)
                        i4 = nc.vector.tensor_tensor(out=vb3, in0=vb3,
                                                     in1=ad_b[:, ka:, :], op=OP.subtract)
                        add_dep_helper(i3.ins, i2.ins, sync=False,
                                       reason="order V_a before V_b")
                        wg = work.tile([128, FREE], BF16, tag="wg", bufs=2)
                        for j in range(KACT):
                            wcol = w_scan[:, t * NJ + j:t * NJ + j + 1]
                            nc.scalar.activation(out=wg[:, j * DV:(j + 1) * DV],
                                                 in_=va[:, j * DV:(j + 1) * DV],
                                                 func=AF.Copy, scale=wcol)
                        for j in range(KACT, NJ):
                            wcol = w_scan[:, t * NJ + j:t * NJ + j + 1]
                            nc.vector.tensor_scalar(out=wg[:, j * DV:(j + 1) * DV],
                                                    in0=vb[:, (j - ka) * DV:(j - ka + 1) * DV],
                                                    scalar1=wcol, scalar2=None,
                                                    op0=OP.mult)
                        nc.vector.tensor_tensor(out=x, in0=x, in1=wg, op=OP.subtract)

            # ---------- phase E: summ/theta/pred ----------
            with tc.tile_pool(name="e_ps", bufs=2, space="PSUM") as e_ps:
                summT = pers.tile([DS + 1, R], BF16)
                nc.sync.dma_start(out=summT[DS:DS + 1, :], in_=ones_d[:, :R])
                for s in range(NSEG):
                    n0, n1 = 512 * s, min(512 * (s + 1), R)
                    sp = e_ps.tile([DS, 512], F32, tag="sp", bufs=2)
                    nc.tensor.matmul(sp[:, :n1 - n0], lhsT=ws1, rhs=readT_lo[:, n0:n1],
                                     start=True, stop=False)
                    nc.tensor.matmul(sp[:, :n1 - n0], lhsT=ws2, rhs=readT_hi[:72, n0:n1],
                                     start=False, stop=False)
                    nc.tensor.matmul(sp[:, :n1 - n0], lhsT=ws3, rhs=qeT[:, n0:n1],
                                     start=False, stop=True)
                    nc.scalar.activation(out=summT[:DS, n0:n1], in_=sp[:, :n1 - n0],
                                         func=AF.Tanh)
                hT = pers.tile([DS + 1, R], BF16)
                nc.sync.dma_start(out=hT[DS:DS + 1, :], in_=ones_d[:, :R])
                for s in range(NSEG):
                    n0, n1 = 512 * s, min(512 * (s + 1), R)
                    hp2 = e_ps.tile([DS, 512], F32, tag="hp2", bufs=2)
                    nc.tensor.matmul(hp2[:, :n1 - n0], lhsT=wa1, rhs=summT[:, n0:n1],
                                     start=True, stop=True)
                    nc.scalar.activation(out=hT[:DS, n0:n1], in_=hp2[:, :n1 - n0],
                                         func=AF.Tanh)
                th_ps = e_ps.tile([128, NCH], F32, tag="th_ps", bufs=1)
                for k in range(NCH):
                    nc.tensor.matmul(th_ps[:, k:k + 1], lhsT=hT[:, 128 * k:128 * (k + 1)],
                                     rhs=wa2, start=True, stop=True)
                pre = pers.tile([128, NCH], F32)
                nc.vector.scalar_tensor_tensor(out=pre, in0=th_ps, scalar=SCALE,
                                               in1=beta_sb, op0=OP.mult, op1=OP.subtract)
                pred_sb = pers.tile([128, NCH], F32)
                nc.scalar.activation(out=pred_sb, in_=pre, func=AF.Sigmoid)
                # pred_sb[p=32u+i, k] -> pred_d[b=i, t=4k+u]
                nc.sync.dma_start(
                    out=pred_d.rearrange("b (k u) -> u b k", u=4),
                    in_=pred_sb[:, :])

    return nc


_NC_CACHE = {}


def _get_nc(T=T_FULL):
    if T not in _NC_CACHE:
        n = build_nc(T=T)
        n.compile()
        _NC_CACHE[T] = n
    return _NC_CACHE[T]


def make_inmaps(q_data, qa_data, q_tab, qa_tab, Mk, Mv0, Ws, bs, Wa1, ba1, Wa2,
                ba2, Wd1, bd1, Wd2, bd2, We, be, Wad, bad, T=T_FULL):
    bf = ml_dtypes.bfloat16
    f32 = np.float32
    q_data = np.asarray(q_data)
    qa_data = np.asarray(qa_data)

    mv0p = np.zeros((MP, DV), f32)
    mv0p[:M] = np.asarray(Mv0, f32)
    x0 = mv0p.reshape(4, NJ, DV)[:, None].repeat(BL, 1).reshape(128, FREE).astype(bf)
    isel = np.tile(np.eye(BL, dtype=f32), (4, 1)).astype(bf)  # (128, 32), g-major
    ident = np.eye(128, dtype=f32)

    cat = np.concatenate
    common = {
        "qtab": np.ascontiguousarray(np.asarray(q_tab, f32)),
        "qatab": np.ascontiguousarray(np.asarray(qa_tab, f32)),
        "x0": x0, "isel": isel, "ident": ident,
        "ones": np.ones((1, BL * T), f32).astype(bf),
        "mkt": np.asarray(Mk, f32).T.copy().astype(bf),
        "we1": np.asarray(We, f32)[:128].astype(bf),
        "we2": cat([np.asarray(We, f32)[128:], np.asarray(be, f32)[None, :]], 0).astype(bf),
        "wad1": np.asarray(Wad, f32)[:128].astype(bf),
        "wad2": cat([np.asarray(Wad, f32)[128:], np.asarray(bad, f32)[None, :]], 0).astype(bf),
        "wd1": cat([np.asarray(Wd1, f32), np.asarray(bd1, f32)[None, :]], 0).astype(bf),
        "wd2": cat([np.asarray(Wd2, f32), np.asarray(bd2, f32)[None, :]], 0).astype(bf),
        "ws1": np.asarray(Ws, f32)[:128].astype(bf),
        "ws2": np.asarray(Ws, f32)[128:200].astype(bf),
        "ws3": cat([np.asarray(Ws, f32)[200:], np.asarray(bs, f32)[None, :]], 0).astype(bf),
        "wa1": cat([np.asarray(Wa1, f32), np.asarray(ba1, f32)[None, :]], 0).astype(bf),
        "wa2": cat([np.asarray(Wa2, f32), np.asarray(ba2, f32)[None, :]], 0).astype(bf),
    }
    in_maps = []
    for c in range(NCORES):
        sl = slice(BL * c, BL * (c + 1))
        in_maps.append(dict(
            common,
            qi=np.ascontiguousarray(q_data[sl, :T].T).reshape(-1).astype(np.int32),
            qai=np.ascontiguousarray(qa_data[sl, :T].T).reshape(-1).astype(np.int32)))
    return in_maps


def kernel(**inputs):
    nc = _get_nc(T_FULL)
    in_maps = make_inmaps(**inputs)
    res = run_bass_kernel_spmd(nc, in_maps, core_ids=list(range(NCORES)), trace=False)
    return np.concatenate([res.results[c]["pred"] for c in range(NCORES)], axis=0)


# revision 17
# speedup vs baseline: 1.1334x; 1.0342x over previous
"""Bass/Trainium2 kernel for nn_DeepIRTModel (DKVMN knowledge tracing).

Strategy: data-parallel over batch (B=256 -> 32 per core on 8 cores).
Per core, sample index r = 32*t + b (t-major). Scan state X = Mv lives in
SBUF as (128 partitions = 32*g+b, free = (j,d)) with slot n = 13*g + j
(52 padded slots, pad weights = 0), d = DV = 200.

Per scan step t (engine assignment):
  DVE:  V = X * er_bcast ; V -= ad_bcast ; wG slices j>=KACT ; X -= wG ;
        W_sel = w_slab_bcast * I_sel_bcast
  ACT:  wG slices j < KACT ; readT PSUM->SBUF copies
  PE:   readT += X_dslice^T @ W_sel_j  (26 matmuls, PSUM accum over j)
Reads use X BEFORE the update (Tile's WAR deps order PE before the X write).
theta/beta/pred are batched matmul chains after the scan.
"""
import sys, types

sys.path.insert(0, '/opt/trn_rl_repo')
import numpy as np
import ml_dtypes


def _install_ntff_hook():
    try:
        import antenv
        if "antenv.axon_hooks" in sys.modules:
            return
        mod = types.ModuleType("antenv.axon_hooks")
        state = {"hook": None}
        mod.set_axon_ntff_profile_hook = lambda h: state.__setitem__("hook", h)
        mod.get_axon_ntff_profile_hook = lambda: state["hook"]
        sys.modules["antenv.axon_hooks"] = mod
        antenv.axon_hooks = mod
        from trn_agent_boot.trn_boot import _ntff_profile_via_ctypes
        mod.set_axon_ntff_profile_hook(_ntff_profile_via_ctypes('/opt/axon/libaxon_pjrt.so'))
    except Exception:
        pass


_install_ntff_hook()

import concourse.bass as bass
import concourse.bacc as bacc
import concourse.mybir as mybir
from concourse.tile import TileContext, add_dep_helper
from concourse.bass_utils import run_bass_kernel_spmd

BF16 = mybir.dt.bfloat16
F32 = mybir.dt.float32
I32 = mybir.dt.int32
AF = mybir.ActivationFunctionType
OP = mybir.AluOpType
AX = mybir.AxisListType

NQ, M, DK, DV, DS = 50000, 50, 50, 200, 50
B, T_FULL = 256, 200
SCALE = 3.0
NCORES = 8
BL = B // NCORES          # 32 batch rows per core
NJ = 13                   # slot groups per partition-subindex g in [0,4)
MP = 4 * NJ               # 52 padded slots
FREE = NJ * DV            # 2600 state free dim
KACT = 6                  # wG slices computed on ACT (j < KACT)


def build_nc(T=T_FULL, Tc=25, sim_safe=False):
    R = BL * T            # samples per core
    NCH = R // 128        # gather chunks of 128 rows
    assert R % 128 == 0
    NSEG = (R + 511) // 512
    nchunks = (T + Tc - 1) // Tc

    nc = bacc.Bacc(trn_type="TRN2")
    # ---- DRAM I/O ----
    qi_d = nc.dram_tensor("qi", [R], I32, kind="ExternalInput")
    qai_d = nc.dram_tensor("qai", [R], I32, kind="ExternalInput")
    qtab_d = nc.dram_tensor("qtab", [NQ + 1, DK], F32, kind="ExternalInput")
    qatab_d = nc.dram_tensor("qatab", [2 * NQ + 1, DV], F32, kind="ExternalInput")
    x0_d = nc.dram_tensor("x0", [128, FREE], BF16, kind="ExternalInput")
    isel_d = nc.dram_tensor("isel", [128, BL], BF16, kind="ExternalInput")
    ident_d = nc.dram_tensor("ident", [128, 128], F32, kind="ExternalInput")
    mkt_d = nc.dram_tensor("mkt", [DK, M], BF16, kind="ExternalInput")
    we1_d = nc.dram_tensor("we1", [128, DV], BF16, kind="ExternalInput")
    we2_d = nc.dram_tensor("we2", [73, DV], BF16, kind="ExternalInput")
    wad1_d = nc.dram_tensor("wad1", [128, DV], BF16, kind="ExternalInput")
    wad2_d = nc.dram_tensor("wad2", [73, DV], BF16, kind="ExternalInput")
    wd1_d = nc.dram_tensor("wd1", [DK + 1, DS], BF16, kind="ExternalInput")
    wd2_d = nc.dram_tensor("wd2", [DS + 1, 1], BF16, kind="ExternalInput")
    ws1_d = nc.dram_tensor("ws1", [128, DS], BF16, kind="ExternalInput")
    ws2_d = nc.dram_tensor("ws2", [72, DS], BF16, kind="ExternalInput")
    ws3_d = nc.dram_tensor("ws3", [DK + 1, DS], BF16, kind="ExternalInput")
    wa1_d = nc.dram_tensor("wa1", [DS + 1, DS], BF16, kind="ExternalInput")
    wa2_d = nc.dram_tensor("wa2", [DS + 1, 1], BF16, kind="ExternalInput")
    ones_d = nc.dram_tensor("ones", [1, BL * T], BF16, kind="ExternalInput")
    pred_d = nc.dram_tensor("pred", [BL, T], F32, kind="ExternalOutput")
    # internal DRAM roundtrip buffers
    er_dram = nc.dram_tensor("er_dram", [R, DV], BF16)
    ad_dram = nc.dram_tensor("ad_dram", [R, DV], BF16)
    w_dram = nc.dram_tensor("w_dram", [R, MP], F32)

    with TileContext(nc) as tc:
        with tc.tile_pool(name="pers", bufs=1) as pers, \
             tc.tile_pool(name="work", bufs=2) as work:

            # ---------- phase A: params + indices ----------
            ident = pers.tile([128, 128], F32)
            nc.sync.dma_start(out=ident, in_=ident_d[:, :])
            iselb = pers.tile([128, BL], BF16)
            nc.sync.dma_start(out=iselb, in_=isel_d[:, :])
            x = pers.tile([128, FREE], BF16)
            nc.sync.dma_start(out=x, in_=x0_d[:, :])
            mkt = pers.tile([DK, M], BF16)
            nc.sync.dma_start(out=mkt, in_=mkt_d[:, :])
            wd1 = pers.tile([DK + 1, DS], BF16)
            nc.sync.dma_start(out=wd1, in_=wd1_d[:, :])
            wd2 = pers.tile([DS + 1, 1], BF16)
            nc.sync.dma_start(out=wd2, in_=wd2_d[:, :])
            ws1 = pers.tile([128, DS], BF16)
            nc.sync.dma_start(out=ws1, in_=ws1_d[:, :])
            ws2 = pers.tile([72, DS], BF16)
            nc.sync.dma_start(out=ws2, in_=ws2_d[:, :])
            ws3 = pers.tile([DK + 1, DS], BF16)
            nc.sync.dma_start(out=ws3, in_=ws3_d[:, :])
            wa1 = pers.tile([DS + 1, DS], BF16)
            nc.sync.dma_start(out=wa1, in_=wa1_d[:, :])
            wa2 = pers.tile([DS + 1, 1], BF16)
            nc.sync.dma_start(out=wa2, in_=wa2_d[:, :])

            qi_sb = pers.tile([128, NCH], I32)
            nc.sync.dma_start(out=qi_sb, in_=qi_d.rearrange("(k p) -> p k", p=128))
            qai_sb = pers.tile([128, NCH], I32)
            nc.sync.dma_start(out=qai_sb, in_=qai_d.rearrange("(k p) -> p k", p=128))

            # persistent across phases
            qeT = pers.tile([DK + 1, R], BF16)
            nc.sync.dma_start(out=qeT[DK:DK + 1, :], in_=ones_d[:, :R])
            beta_sb = pers.tile([128, NCH], F32)
            w_scan = pers.tile([128, T * NJ], F32)
            w_scanb = pers.tile([128, T * NJ], BF16)
            readT_lo = pers.tile([128, R], BF16)
            readT_hi = pers.tile([72, R], BF16)
            zpad = pers.tile([128, 2], F32)
            nc.vector.memset(zpad, 0.0)

            # ---------- phases A2-C: gather, transpose, w/er/ad/beta ----------
            with tc.tile_pool(name="ac_sb", bufs=1) as ac_sb, \
                 tc.tile_pool(name="ac_ps", bufs=2, space="PSUM") as ac_ps:
                we1 = ac_sb.tile([128, DV], BF16)
                nc.sync.dma_start(out=we1, in_=we1_d[:, :])
                we2 = ac_sb.tile([73, DV], BF16)
                nc.sync.dma_start(out=we2, in_=we2_d[:, :])
                wad1 = ac_sb.tile([128, DV], BF16)
                nc.sync.dma_start(out=wad1, in_=wad1_d[:, :])
                wad2 = ac_sb.tile([73, DV], BF16)
                nc.sync.dma_start(out=wad2, in_=wad2_d[:, :])
                qaeT_lo = ac_sb.tile([128, R], BF16)
                qaeT_hi = ac_sb.tile([73, R], BF16)
                nc.sync.dma_start(out=qaeT_hi[72:73, :], in_=ones_d[:, :R])
                h2T = ac_sb.tile([DS + 1, R], BF16)
                nc.sync.dma_start(out=h2T[DS:DS + 1, :], in_=ones_d[:, :R])

                w_w_insts = []
                er_w_insts = {}
                ad_w_insts = {}
                for k in range(NCH):
                    qe_g = ac_sb.tile([128, DK], F32, tag="qe_g", bufs=3)
                    nc.gpsimd.indirect_dma_start(
                        out=qe_g, out_offset=None, in_=qtab_d[:, :],
                        in_offset=bass.IndirectOffsetOnAxis(ap=qi_sb[:, k:k + 1], axis=0))
                    qae_g = ac_sb.tile([128, DV], F32, tag="qae_g", bufs=3)
                    nc.gpsimd.indirect_dma_start(
                        out=qae_g, out_offset=None, in_=qatab_d[:, :],
                        in_offset=bass.IndirectOffsetOnAxis(ap=qai_sb[:, k:k + 1], axis=0))
                    pt = ac_ps.tile([128, 128], F32, tag="pt", bufs=2)
                    nc.tensor.transpose(out=pt[:DK, :], in_=qe_g, identity=ident)
                    nc.scalar.copy(qeT[:DK, 128 * k:128 * (k + 1)], pt[:DK, :])
                    pt2 = ac_ps.tile([128, 128], F32, tag="pt", bufs=2)
                    nc.tensor.transpose(out=pt2, in_=qae_g[:, :128], identity=ident)
                    nc.scalar.copy(qaeT_lo[:, 128 * k:128 * (k + 1)], pt2)
                    pt3 = ac_ps.tile([128, 128], F32, tag="pt", bufs=2)
                    nc.tensor.transpose(out=pt3[:72, :], in_=qae_g[:, 128:200], identity=ident)
                    nc.scalar.copy(qaeT_hi[:72, 128 * k:128 * (k + 1)], pt3[:72, :])

                GK = 7
                for g0 in range(0, NCH, GK):
                    grp = range(g0, min(g0 + GK, NCH))
                    for k in grp:
                        lg = ac_ps.tile([128, M], F32, tag="lg", bufs=2)
                        nc.tensor.matmul(lg, lhsT=qeT[:DK, 128 * k:128 * (k + 1)],
                                         rhs=mkt, start=True, stop=True)
                        ex = ac_sb.tile([128, M], F32, tag="ex", bufs=3)
                        nc.scalar.activation(out=ex, in_=lg, func=AF.Exp)
                        sm = ac_sb.tile([128, 1], F32, tag="sm", bufs=3)
                        nc.vector.reduce_sum(sm, ex, axis=AX.X)
                        rc = ac_sb.tile([128, 1], F32, tag="rc", bufs=3)
                        nc.vector.reciprocal(rc, sm)
                        wn = ac_sb.tile([128, M], F32, tag="wn", bufs=3)
                        nc.vector.tensor_scalar(out=wn, in0=ex, scalar1=rc[:, 0:1],
                                                scalar2=None, op0=OP.mult)
                        w_w_insts.append(nc.sync.dma_start(
                            out=w_dram[128 * k:128 * (k + 1), :M], in_=wn))
                        w_w_insts.append(nc.sync.dma_start(
                            out=w_dram[128 * k:128 * (k + 1), M:MP], in_=zpad[:, :2]))
                    for k in grp:
                        ep = ac_ps.tile([128, DV], F32, tag="eap", bufs=2, name="ep")
                        nc.tensor.matmul(ep, lhsT=qaeT_lo[:, 128 * k:128 * (k + 1)],
                                         rhs=we1, start=True, stop=False)
                        nc.tensor.matmul(ep, lhsT=qaeT_hi[:, 128 * k:128 * (k + 1)],
                                         rhs=we2, start=False, stop=True)
                        ero = ac_sb.tile([128, DV], BF16, tag="ero", bufs=3)
                        nc.scalar.activation(out=ero, in_=ep, func=AF.Sigmoid)
                        er_w_insts[k] = nc.sync.dma_start(
                            out=er_dram[128 * k:128 * (k + 1), :], in_=ero)
                    for k in grp:
                        ap_ = ac_ps.tile([128, DV], F32, tag="eap", bufs=2, name="ap_")
                        nc.tensor.matmul(ap_, lhsT=qaeT_lo[:, 128 * k:128 * (k + 1)],
                                         rhs=wad1, start=True, stop=False)
                        nc.tensor.matmul(ap_, lhsT=qaeT_hi[:, 128 * k:128 * (k + 1)],
                                         rhs=wad2, start=False, stop=True)
                        ado = ac_sb.tile([128, DV], BF16, tag="ado", bufs=3)
                        nc.scalar.activation(out=ado, in_=ap_, func=AF.Tanh)
                        ad_w_insts[k] = nc.gpsimd.dma_start(
                            out=ad_dram[128 * k:128 * (k + 1), :], in_=ado)

                # beta chain
                for s in range(NSEG):
                    n0, n1 = 512 * s, min(512 * (s + 1), R)
                    hp = ac_ps.tile([DS, 512], F32, tag="hp", bufs=1)
                    nc.tensor.matmul(hp[:, :n1 - n0], lhsT=wd1, rhs=qeT[:, n0:n1],
                                     start=True, stop=True)
                    nc.scalar.activation(out=h2T[:DS, n0:n1], in_=hp[:, :n1 - n0],
                                         func=AF.Tanh)
                beta_ps = ac_ps.tile([128, NCH], F32, tag="beta_ps", bufs=1)
                for k in range(NCH):
                    nc.tensor.matmul(beta_ps[:, k:k + 1],
                                     lhsT=h2T[:, 128 * k:128 * (k + 1)],
                                     rhs=wd2, start=True, stop=True)
                nc.scalar.copy(beta_sb, beta_ps)

                # w_scan load: (128=(4b+g), (t,13)) from w_dram[(32t+b), 13g+j]


            # ---------- phase D: the scan ----------
            er_src = er_dram.rearrange("(t b) d -> t b d", b=BL)
            ad_src = ad_dram.rearrange("(t b) d -> t b d", b=BL)
            with tc.tile_pool(name="d_ps", bufs=2, space="PSUM") as d_ps:
                for ch in range(nchunks):
                    t0, t1 = Tc * ch, min(Tc * (ch + 1), T)
                    tl = t1 - t0
                    er_ch = work.tile([128, Tc * DV], BF16, tag="er_ch", bufs=2)
                    ad_ch = work.tile([128, Tc * DV], BF16, tag="ad_ch", bufs=2)
                    if sim_safe:
                        nc.vector.memset(er_ch, 0.0)
                        nc.vector.memset(ad_ch, 0.0)
                    ks = range(t0 // 4, (t1 + 3) // 4)
                    for g in range(4):
                        ldi = nc.sync.dma_start(
                            out=er_ch[32 * g:32 * (g + 1), :tl * DV].rearrange(
                                "b (t d) -> b t d", d=DV),
                            in_=er_src[t0:t1, :, :].rearrange("t b d -> b t d"))
                        for k in ks:
                            add_dep_helper(ldi.ins, er_w_insts[k].ins, reason="er roundtrip")
                        lda = nc.gpsimd.dma_start(
                            out=ad_ch[32 * g:32 * (g + 1), :tl * DV].rearrange(
                                "b (t d) -> b t d", d=DV),
                            in_=ad_src[t0:t1, :, :].rearrange("t b d -> b t d"))
                        for k in ks:
                            add_dep_helper(lda.ins, ad_w_insts[k].ins, reason="ad roundtrip")

                    if sim_safe and ch == 0:
                        nc.vector.memset(w_scan, 0.0)
                    for g in range(4):
                        src = w_dram[:, NJ * g:NJ * (g + 1)].rearrange(
                            "(t b) j -> b t j", b=BL)[:, t0:t1, :]
                        dst = w_scan[32 * g:32 * (g + 1),
                                     t0 * NJ:t1 * NJ].rearrange(
                            "b (t j) -> b t j", j=NJ)
                        ldw = nc.sync.dma_start(out=dst, in_=src)
                        for k in ks:
                            add_dep_helper(ldw.ins, w_w_insts[2 * k].ins,
                                           reason="w roundtrip")
                            add_dep_helper(ldw.ins, w_w_insts[2 * k + 1].ins,
                                           reason="w pad roundtrip")
                    nc.vector.tensor_copy(w_scanb[:, t0 * NJ:t1 * NJ],
                                          w_scan[:, t0 * NJ:t1 * NJ])
                    for tt in range(tl):
                        t = t0 + tt
                        er_t = er_ch[:, tt * DV:(tt + 1) * DV]
                        ad_t = ad_ch[:, tt * DV:(tt + 1) * DV]
                        er_b = er_t.rearrange("p (o d) -> p o d", o=1).to_broadcast(
                            [128, NJ, DV])
                        ad_b = ad_t.rearrange("p (o d) -> p o d", o=1).to_broadcast(
                            [128, NJ, DV])

                        wsel = work.tile([128, NJ * BL], BF16, tag="wsel", bufs=2)
                        nc.gpsimd.tensor_tensor(
                            out=wsel.rearrange("p (j b) -> p j b", j=NJ),
                            in0=w_scanb[:, t * NJ:(t + 1) * NJ].rearrange(
                                "p (j o) -> p j o", o=1).to_broadcast([128, NJ, BL]),
                            in1=iselb.rearrange("p (o b) -> p o b", o=1).to_broadcast(
                                [128, NJ, BL]),
                            op=OP.mult)

                        # PE: readT (uses X before this step's update)
                        rlo = d_ps.tile([128, BL], F32, tag="rlo", bufs=2)
                        rhi = d_ps.tile([72, BL], F32, tag="rhi", bufs=2)
                        for j in range(NJ):
                            nc.tensor.matmul(rlo, lhsT=x[:, j * DV:j * DV + 128],
                                             rhs=wsel[:, j * BL:(j + 1) * BL],
                                             start=(j == 0), stop=(j == NJ - 1))
                        for j in range(NJ):
                            nc.tensor.matmul(rhi, lhsT=x[:, j * DV + 128:(j + 1) * DV],
                                             rhs=wsel[:, j * BL:(j + 1) * BL],
                                             start=(j == 0), stop=(j == NJ - 1))
                        nc.scalar.copy(readT_lo[:, BL * t:BL * (t + 1)], rlo)
                        nc.scalar.copy(readT_hi[:, BL * t:BL * (t + 1)], rhi[:72, :])

                        # DVE: V = X*er_b ; V -= ad_b (split so ACT starts early)
                        ka = KACT
                        kb = NJ - KACT
                        va = work.tile([128, ka * DV], BF16, tag="va", bufs=2)
                        vb = work.tile([128, kb * DV], BF16, tag="vb", bufs=2)
                        va3 = va.rearrange("p (j d) -> p j d", j=ka)
                        vb3 = vb.rearrange("p (j d) -> p j d", j=kb)
                        x3 = x.rearrange("p (j d) -> p j d", j=NJ)
                        i1 = nc.vector.tensor_tensor(out=va3, in0=x3[:, :ka, :],
                                                     in1=er_b[:, :ka, :], op=OP.mult)
                        i2 = nc.vector.tensor_tensor(out=va3, in0=va3,
                                                     in1=ad_b[:, :ka, :], op=OP.subtract)
                        i3 = nc.vector.tensor_tensor(out=vb3, in0=x3[:, ka:, :],
                                                     in1=er_b[:, ka:, :], op=OP.mult)
                        kb0 = kb - (kb % 2)
                        i4 = nc.vector.tensor_tensor(out=vb3[:, :kb0, :], in0=vb3[:, :kb0, :],
                                                     in1=ad_b[:, ka:ka + kb0, :], op=OP.subtract)
                        if kb > kb0:
                            nc.vector.tensor_tensor(out=vb3[:, kb0:, :], in0=vb3[:, kb0:, :],
                                                    in1=ad_b[:, ka + kb0:, :], op=OP.subtract)
                        add_dep_helper(i3.ins, i2.ins, sync=False,
                                       reason="order V_a before V_b")
                        wg = work.tile([128, FREE], BF16, tag="wg", bufs=2)
                        for j in range(KACT):
                            wcol = w_scan[:, t * NJ + j:t * NJ + j + 1]
                            nc.scalar.activation(out=wg[:, j * DV:(j + 1) * DV],
                                                 in_=va[:, j * DV:(j + 1) * DV],
                                                 func=AF.Copy, scale=wcol)
                        for j in range(KACT, NJ):
                            wcol = w_scan[:, t * NJ + j:t * NJ + j + 1]
                            nc.vector.tensor_scalar(out=wg[:, j * DV:(j + 1) * DV],
                                                    in0=vb[:, (j - ka) * DV:(j - ka + 1) * DV],
                                                    scalar1=wcol, scalar2=None,
                                                    op0=OP.mult)
                        nc.vector.tensor_tensor(out=x, in0=x, in1=wg, op=OP.subtract)

            # ---------- phase E: summ/theta/pred ----------
            with tc.tile_pool(name="e_ps", bufs=2, space="PSUM") as e_ps:
                summT = pers.tile([DS + 1, R], BF16)
                nc.sync.dma_start(out=summT[DS:DS + 1, :], in_=ones_d[:, :R])
                for s in range(NSEG):
                    n0, n1 = 512 * s, min(512 * (s + 1), R)
                    sp = e_ps.tile([DS, 512], F32, tag="sp", bufs=2)
                    nc.tensor.matmul(sp[:, :n1 - n0], lhsT=ws1, rhs=readT_lo[:, n0:n1],
                                     start=True, stop=False)
                    nc.tensor.matmul(sp[:, :n1 - n0], lhsT=ws2, rhs=readT_hi[:72, n0:n1],
                                     start=False, stop=False)
                    nc.tensor.matmul(sp[:, :n1 - n0], lhsT=ws3, rhs=qeT[:, n0:n1],
                                     start=False, stop=True)
                    nc.scalar.activation(out=summT[:DS, n0:n1], in_=sp[:, :n1 - n0],
                                         func=AF.Tanh)
                hT = pers.tile([DS + 1, R], BF16)
                nc.sync.dma_start(out=hT[DS:DS + 1, :], in_=ones_d[:, :R])
                for s in range(NSEG):
                    n0, n1 = 512 * s, min(512 * (s + 1), R)
                    hp2 = e_ps.tile([DS, 512], F32, tag="hp2", bufs=2)
                    nc.tensor.matmul(hp2[:, :n1 - n0], lhsT=wa1, rhs=summT[:, n0:n1],
                                     start=True, stop=True)
                    nc.scalar.activation(out=hT[:DS, n0:n1], in_=hp2[:, :n1 - n0],
                                         func=AF.Tanh)
                th_ps = e_ps.tile([128, NCH], F32, tag="th_ps", bufs=1)
                for k in range(NCH):
                    nc.tensor.matmul(th_ps[:, k:k + 1], lhsT=hT[:, 128 * k:128 * (k + 1)],
                                     rhs=wa2, start=True, stop=True)
                pre = pers.tile([128, NCH], F32)
                nc.vector.scalar_tensor_tensor(out=pre, in0=th_ps, scalar=SCALE,
                                               in1=beta_sb, op0=OP.mult, op1=OP.subtract)
                pred_sb = pers.tile([128, NCH], F32)
                nc.scalar.activation(out=pred_sb, in_=pre, func=AF.Sigmoid)
                # pred_sb[p=32u+i, k] -> pred_d[b=i, t=4k+u]
                nc.sync.dma_start(
                    out=pred_d.rearrange("b (k u) -> u b k", u=4),
                    in_=pred_sb[:, :])

    return nc


_NC_CACHE = {}


def _get_nc(T=T_FULL):
    if T not in _NC_CACHE:
        n = build_nc(T=T)
        n.compile()
        _NC_CACHE[T] = n
    return _NC_CACHE[T]


def make_inmaps(q_data, qa_data, q_tab, qa_tab, Mk, Mv0, Ws, bs, Wa1, ba1, Wa2,
                ba2, Wd1, bd1, Wd2, bd2, We, be, Wad, bad, T=T_FULL):
    bf = ml_dtypes.bfloat16
    f32 = np.float32
    q_data = np.asarray(q_data)
    qa_data = np.asarray(qa_data)

    mv0p = np.zeros((MP, DV), f32)
    mv0p[:M] = np.asarray(Mv0, f32)
    x0 = mv0p.reshape(4, NJ, DV)[:, None].repeat(BL, 1).reshape(128, FREE).astype(bf)
    isel = np.tile(np.eye(BL, dtype=f32), (4, 1)).astype(bf)  # (128, 32), g-major
    ident = np.eye(128, dtype=f32)

    cat = np.concatenate
    common = {
        "qtab": np.ascontiguousarray(np.asarray(q_tab, f32)),
        "qatab": np.ascontiguousarray(np.asarray(qa_tab, f32)),
        "x0": x0, "isel": isel, "ident": ident,
        "ones": np.ones((1, BL * T), f32).astype(bf),
        "mkt": np.asarray(Mk, f32).T.copy().astype(bf),
        "we1": np.asarray(We, f32)[:128].astype(bf),
        "we2": cat([np.asarray(We, f32)[128:], np.asarray(be, f32)[None, :]], 0).astype(bf),
        "wad1": np.asarray(Wad, f32)[:128].astype(bf),
        "wad2": cat([np.asarray(Wad, f32)[128:], np.asarray(bad, f32)[None, :]], 0).astype(bf),
        "wd1": cat([np.asarray(Wd1, f32), np.asarray(bd1, f32)[None, :]], 0).astype(bf),
        "wd2": cat([np.asarray(Wd2, f32), np.asarray(bd2, f32)[None, :]], 0).astype(bf),
        "ws1": np.asarray(Ws, f32)[:128].astype(bf),
        "ws2": np.asarray(Ws, f32)[128:200].astype(bf),
        "ws3": cat([np.asarray(Ws, f32)[200:], np.asarray(bs, f32)[None, :]], 0).astype(bf),
        "wa1": cat([np.asarray(Wa1, f32), np.asarray(ba1, f32)[None, :]], 0).astype(bf),
        "wa2": cat([np.asarray(Wa2, f32), np.asarray(ba2, f32)[None, :]], 0).astype(bf),
    }
    in_maps = []
    for c in range(NCORES):
        sl = slice(BL * c, BL * (c + 1))
        in_maps.append(dict(
            common,
            qi=np.ascontiguousarray(q_data[sl, :T].T).reshape(-1).astype(np.int32),
            qai=np.ascontiguousarray(qa_data[sl, :T].T).reshape(-1).astype(np.int32)))
    return in_maps


def kernel(**inputs):
    nc = _get_nc(T_FULL)
    in_maps = make_inmaps(**inputs)
    res = run_bass_kernel_spmd(nc, in_maps, core_ids=list(range(NCORES)), trace=False)
    return np.concatenate([res.results[c]["pred"] for c in range(NCORES)], axis=0)


# revision 19
# speedup vs baseline: 1.2087x; 1.0664x over previous
"""Bass/Trainium2 kernel for nn_DeepIRTModel (DKVMN knowledge tracing).

Strategy: data-parallel over batch (B=256 -> 32 per core on 8 cores).
Per core, sample index r = 32*t + b (t-major). Scan state X = Mv lives in
SBUF as (128 partitions = 32*g+b, free = (j,d)) with slot n = 13*g + j
(52 padded slots, pad weights = 0), d = DV = 200.

Per scan step t (engine assignment):
  DVE:  V = X * er_bcast ; V -= ad_bcast ; wG slices j>=KACT ; X -= wG ;
        W_sel = w_slab_bcast * I_sel_bcast
  ACT:  wG slices j < KACT ; readT PSUM->SBUF copies
  PE:   readT += X_dslice^T @ W_sel_j  (26 matmuls, PSUM accum over j)
Reads use X BEFORE the update (Tile's WAR deps order PE before the X write).
theta/beta/pred are batched matmul chains after the scan.
"""
import sys, types

sys.path.insert(0, '/opt/trn_rl_repo')
import numpy as np
import ml_dtypes


def _install_ntff_hook():
    try:
        import antenv
        if "antenv.axon_hooks" in sys.modules:
            return
        mod = types.ModuleType("antenv.axon_hooks")
        state = {"hook": None}
        mod.set_axon_ntff_profile_hook = lambda h: state.__setitem__("hook", h)
        mod.get_axon_ntff_profile_hook = lambda: state["hook"]
        sys.modules["antenv.axon_hooks"] = mod
        antenv.axon_hooks = mod
        from trn_agent_boot.trn_boot import _ntff_profile_via_ctypes
        mod.set_axon_ntff_profile_hook(_ntff_profile_via_ctypes('/opt/axon/libaxon_pjrt.so'))
    except Exception:
        pass


_install_ntff_hook()

import concourse.bass as bass
import concourse.bacc as bacc
import concourse.mybir as mybir
from concourse.tile import TileContext, add_dep_helper
from concourse.bass_utils import run_bass_kernel_spmd

BF16 = mybir.dt.bfloat16
F32 = mybir.dt.float32
I32 = mybir.dt.int32
AF = mybir.ActivationFunctionType
OP = mybir.AluOpType
AX = mybir.AxisListType

NQ, M, DK, DV, DS = 50000, 50, 50, 200, 50
B, T_FULL = 256, 200
SCALE = 3.0
NCORES = 8
BL = B // NCORES          # 32 batch rows per core
NJ = 13                   # slot groups per partition-subindex g in [0,4)
MP = 4 * NJ               # 52 padded slots
FREE = NJ * DV            # 2600 state free dim
KACT = 6                  # wG slices computed on ACT (j < KACT)


def build_nc(T=T_FULL, Tc=25, sim_safe=False):
    R = BL * T            # samples per core
    NCH = R // 128        # gather chunks of 128 rows
    assert R % 128 == 0
    NSEG = (R + 511) // 512
    nchunks = (T + Tc - 1) // Tc

    nc = bacc.Bacc(trn_type="TRN2")
    # ---- DRAM I/O ----
    qi_d = nc.dram_tensor("qi", [R], I32, kind="ExternalInput")
    qai_d = nc.dram_tensor("qai", [R], I32, kind="ExternalInput")
    qtab_d = nc.dram_tensor("qtab", [NQ + 1, DK], F32, kind="ExternalInput")
    qatab_d = nc.dram_tensor("qatab", [2 * NQ + 1, DV], F32, kind="ExternalInput")
    x0_d = nc.dram_tensor("x0", [128, FREE], BF16, kind="ExternalInput")
    isel_d = nc.dram_tensor("isel", [128, BL], BF16, kind="ExternalInput")
    ident_d = nc.dram_tensor("ident", [128, 128], F32, kind="ExternalInput")
    mkt_d = nc.dram_tensor("mkt", [DK, M], BF16, kind="ExternalInput")
    we1_d = nc.dram_tensor("we1", [128, DV], BF16, kind="ExternalInput")
    we2_d = nc.dram_tensor("we2", [73, DV], BF16, kind="ExternalInput")
    wad1_d = nc.dram_tensor("wad1", [128, DV], BF16, kind="ExternalInput")
    wad2_d = nc.dram_tensor("wad2", [73, DV], BF16, kind="ExternalInput")
    wd1_d = nc.dram_tensor("wd1", [DK + 1, DS], BF16, kind="ExternalInput")
    wd2_d = nc.dram_tensor("wd2", [DS + 1, 1], BF16, kind="ExternalInput")
    ws1_d = nc.dram_tensor("ws1", [128, DS], BF16, kind="ExternalInput")
    ws2_d = nc.dram_tensor("ws2", [72, DS], BF16, kind="ExternalInput")
    ws3_d = nc.dram_tensor("ws3", [DK + 1, DS], BF16, kind="ExternalInput")
    wa1_d = nc.dram_tensor("wa1", [DS + 1, DS], BF16, kind="ExternalInput")
    wa2_d = nc.dram_tensor("wa2", [DS + 1, 1], BF16, kind="ExternalInput")
    ones_d = nc.dram_tensor("ones", [1, BL * T], BF16, kind="ExternalInput")
    pred_d = nc.dram_tensor("pred", [BL, T], F32, kind="ExternalOutput")
    # internal DRAM roundtrip buffers
    er_dram = nc.dram_tensor("er_dram", [R, DV], BF16)
    ad_dram = nc.dram_tensor("ad_dram", [R, DV], BF16)
    w_dram = nc.dram_tensor("w_dram", [R, MP], F32)

    with TileContext(nc) as tc:
        with tc.tile_pool(name="pers", bufs=1) as pers, \
             tc.tile_pool(name="work", bufs=2) as work:

            # ---------- phase A: params + indices ----------
            ident = pers.tile([128, 128], F32)
            nc.sync.dma_start(out=ident, in_=ident_d[:, :])
            iselb = pers.tile([128, BL], BF16)
            nc.sync.dma_start(out=iselb, in_=isel_d[:, :])
            x = pers.tile([128, FREE], BF16)
            nc.sync.dma_start(out=x, in_=x0_d[:, :])
            mkt = pers.tile([DK, M], BF16)
            nc.sync.dma_start(out=mkt, in_=mkt_d[:, :])
            wd1 = pers.tile([DK + 1, DS], BF16)
            nc.sync.dma_start(out=wd1, in_=wd1_d[:, :])
            wd2 = pers.tile([DS + 1, 1], BF16)
            nc.sync.dma_start(out=wd2, in_=wd2_d[:, :])
            ws1 = pers.tile([128, DS], BF16)
            nc.sync.dma_start(out=ws1, in_=ws1_d[:, :])
            ws2 = pers.tile([72, DS], BF16)
            nc.sync.dma_start(out=ws2, in_=ws2_d[:, :])
            ws3 = pers.tile([DK + 1, DS], BF16)
            nc.sync.dma_start(out=ws3, in_=ws3_d[:, :])
            wa1 = pers.tile([DS + 1, DS], BF16)
            nc.sync.dma_start(out=wa1, in_=wa1_d[:, :])
            wa2 = pers.tile([DS + 1, 1], BF16)
            nc.sync.dma_start(out=wa2, in_=wa2_d[:, :])

            qi_sb = pers.tile([128, NCH], I32)
            nc.sync.dma_start(out=qi_sb, in_=qi_d.rearrange("(k p) -> p k", p=128))
            qai_sb = pers.tile([128, NCH], I32)
            nc.sync.dma_start(out=qai_sb, in_=qai_d.rearrange("(k p) -> p k", p=128))

            # persistent across phases
            qeT = pers.tile([DK + 1, R], BF16)
            nc.sync.dma_start(out=qeT[DK:DK + 1, :], in_=ones_d[:, :R])
            beta_sb = pers.tile([128, NCH], F32)
            w_scan = pers.tile([128, T * NJ], F32)
            w_scanb = pers.tile([128, T * NJ], BF16)
            readT_lo = pers.tile([128, R], BF16)
            readT_hi = pers.tile([72, R], BF16)
            zpad = pers.tile([128, 2], F32)
            nc.vector.memset(zpad, 0.0)

            # ---------- interleaved phase C groups + scan chunks ----------
            er_src = er_dram.rearrange("(t b) d -> t b d", b=BL)
            ad_src = ad_dram.rearrange("(t b) d -> t b d", b=BL)
            w_w_insts = {}
            er_w_insts = {}
            ad_w_insts = {}

            with tc.tile_pool(name="ac_sb", bufs=1) as ac_sb, \
                 tc.tile_pool(name="ac_ps", bufs=2, space="PSUM") as ac_ps, \
                 tc.tile_pool(name="d_ps", bufs=2, space="PSUM") as d_ps:
                we1 = ac_sb.tile([128, DV], BF16)
                nc.sync.dma_start(out=we1, in_=we1_d[:, :])
                we2 = ac_sb.tile([73, DV], BF16)
                nc.sync.dma_start(out=we2, in_=we2_d[:, :])
                wad1 = ac_sb.tile([128, DV], BF16)
                nc.sync.dma_start(out=wad1, in_=wad1_d[:, :])
                wad2 = ac_sb.tile([73, DV], BF16)
                nc.sync.dma_start(out=wad2, in_=wad2_d[:, :])
                qaeT_lo = ac_sb.tile([128, R], BF16)
                qaeT_hi = ac_sb.tile([73, R], BF16)
                nc.sync.dma_start(out=qaeT_hi[72:73, :], in_=ones_d[:, :R])

                def pre_group(klo, khi):
                    for k in range(klo, khi):
                        qe_g = ac_sb.tile([128, DK], F32, tag="qe_g", bufs=3)
                        nc.gpsimd.indirect_dma_start(
                            out=qe_g, out_offset=None, in_=qtab_d[:, :],
                            in_offset=bass.IndirectOffsetOnAxis(
                                ap=qi_sb[:, k:k + 1], axis=0))
                        qae_g = ac_sb.tile([128, DV], F32, tag="qae_g", bufs=3)
                        nc.gpsimd.indirect_dma_start(
                            out=qae_g, out_offset=None, in_=qatab_d[:, :],
                            in_offset=bass.IndirectOffsetOnAxis(
                                ap=qai_sb[:, k:k + 1], axis=0))
                        pt = ac_ps.tile([128, 128], F32, tag="pt", bufs=1)
                        nc.tensor.transpose(out=pt[:DK, :], in_=qe_g, identity=ident)
                        nc.scalar.copy(qeT[:DK, 128 * k:128 * (k + 1)], pt[:DK, :])
                        pt2 = ac_ps.tile([128, 128], F32, tag="pt", bufs=1)
                        nc.tensor.transpose(out=pt2, in_=qae_g[:, :128], identity=ident)
                        nc.scalar.copy(qaeT_lo[:, 128 * k:128 * (k + 1)], pt2)
                        pt3 = ac_ps.tile([128, 128], F32, tag="pt", bufs=1)
                        nc.tensor.transpose(out=pt3[:72, :], in_=qae_g[:, 128:200],
                                            identity=ident)
                        nc.scalar.copy(qaeT_hi[:72, 128 * k:128 * (k + 1)], pt3[:72, :])
                    for k in range(klo, khi):
                        lg = ac_ps.tile([128, M], F32, tag="lg", bufs=1)
                        nc.tensor.matmul(lg, lhsT=qeT[:DK, 128 * k:128 * (k + 1)],
                                         rhs=mkt, start=True, stop=True)
                        ex = ac_sb.tile([128, M], F32, tag="ex", bufs=3)
                        nc.scalar.activation(out=ex, in_=lg, func=AF.Exp)
                        sm = ac_sb.tile([128, 1], F32, tag="sm", bufs=3)
                        nc.vector.reduce_sum(sm, ex, axis=AX.X)
                        rc = ac_sb.tile([128, 1], F32, tag="rc", bufs=3)
                        nc.vector.reciprocal(rc, sm)
                        wn = ac_sb.tile([128, M], F32, tag="wn", bufs=3)
                        nc.vector.tensor_scalar(out=wn, in0=ex, scalar1=rc[:, 0:1],
                                                scalar2=None, op0=OP.mult)
                        w_w_insts[2 * k] = nc.sync.dma_start(
                            out=w_dram[128 * k:128 * (k + 1), :M], in_=wn)
                        w_w_insts[2 * k + 1] = nc.sync.dma_start(
                            out=w_dram[128 * k:128 * (k + 1), M:MP], in_=zpad[:, :2])
                    for k in range(klo, khi):
                        ep = ac_ps.tile([128, DV], F32, tag="eap", bufs=2, name="ep")
                        nc.tensor.matmul(ep, lhsT=qaeT_lo[:, 128 * k:128 * (k + 1)],
                                         rhs=we1, start=True, stop=False)
                        nc.tensor.matmul(ep, lhsT=qaeT_hi[:, 128 * k:128 * (k + 1)],
                                         rhs=we2, start=False, stop=True)
                        ero = ac_sb.tile([128, DV], BF16, tag="ero", bufs=3)
                        nc.scalar.activation(out=ero, in_=ep, func=AF.Sigmoid)
                        er_w_insts[k] = nc.sync.dma_start(
                            out=er_dram[128 * k:128 * (k + 1), :], in_=ero)
                    for k in range(klo, khi):
                        ap_ = ac_ps.tile([128, DV], F32, tag="eap", bufs=2, name="ap_")
                        nc.tensor.matmul(ap_, lhsT=qaeT_lo[:, 128 * k:128 * (k + 1)],
                                         rhs=wad1, start=True, stop=False)
                        nc.tensor.matmul(ap_, lhsT=qaeT_hi[:, 128 * k:128 * (k + 1)],
                                         rhs=wad2, start=False, stop=True)
                        ado = ac_sb.tile([128, DV], BF16, tag="ado", bufs=3)
                        nc.scalar.activation(out=ado, in_=ap_, func=AF.Tanh)
                        ad_w_insts[k] = nc.sync.dma_start(
                            out=ad_dram[128 * k:128 * (k + 1), :], in_=ado)

                kdone = 0
                for ch in range(nchunks):
                    t0, t1 = Tc * ch, min(Tc * (ch + 1), T)
                    tl = t1 - t0
                    knext = NCH if ch == nchunks - 1 else min(NCH, (t1 + 3) // 4)
                    pre_group(kdone, knext)
                    kdone = knext

                    er_ch = work.tile([128, Tc * DV], BF16, tag="er_ch", bufs=2)
                    ad_ch = work.tile([128, Tc * DV], BF16, tag="ad_ch", bufs=2)
                    if sim_safe:
                        nc.vector.memset(er_ch, 0.0)
                        nc.vector.memset(ad_ch, 0.0)
                    ks = range(t0 // 4, (t1 + 3) // 4)
                    for g in range(4):
                        ldi = nc.sync.dma_start(
                            out=er_ch[32 * g:32 * (g + 1), :tl * DV].rearrange(
                                "b (t d) -> b t d", d=DV),
                            in_=er_src[t0:t1, :, :].rearrange("t b d -> b t d"))
                        for k in ks:
                            add_dep_helper(ldi.ins, er_w_insts[k].ins,
                                           reason="er roundtrip")
                        lda = nc.gpsimd.dma_start(
                            out=ad_ch[32 * g:32 * (g + 1), :tl * DV].rearrange(
                                "b (t d) -> b t d", d=DV),
                            in_=ad_src[t0:t1, :, :].rearrange("t b d -> b t d"))
                        for k in ks:
                            add_dep_helper(lda.ins, ad_w_insts[k].ins,
                                           reason="ad roundtrip")
                    if sim_safe and ch == 0:
                        nc.vector.memset(w_scan, 0.0)
                    for g in range(4):
                        src = w_dram[:, NJ * g:NJ * (g + 1)].rearrange(
                            "(t b) j -> b t j", b=BL)[:, t0:t1, :]
                        dst = w_scan[32 * g:32 * (g + 1),
                                     t0 * NJ:t1 * NJ].rearrange(
                            "b (t j) -> b t j", j=NJ)
                        ldw = nc.sync.dma_start(out=dst, in_=src)
                        for k in ks:
                            add_dep_helper(ldw.ins, w_w_insts[2 * k].ins,
                                           reason="w roundtrip")
                            add_dep_helper(ldw.ins, w_w_insts[2 * k + 1].ins,
                                           reason="w pad roundtrip")
                    nc.vector.tensor_copy(w_scanb[:, t0 * NJ:t1 * NJ],
                                          w_scan[:, t0 * NJ:t1 * NJ])

                    for tt in range(tl):
                        t = t0 + tt
                        er_t = er_ch[:, tt * DV:(tt + 1) * DV]
                        ad_t = ad_ch[:, tt * DV:(tt + 1) * DV]
                        er_b = er_t.rearrange("p (o d) -> p o d", o=1).to_broadcast(
                            [128, NJ, DV])
                        ad_b = ad_t.rearrange("p (o d) -> p o d", o=1).to_broadcast(
                            [128, NJ, DV])

                        wsel = work.tile([128, NJ * BL], BF16, tag="wsel", bufs=2)
                        nc.gpsimd.tensor_tensor(
                            out=wsel.rearrange("p (j b) -> p j b", j=NJ),
                            in0=w_scanb[:, t * NJ:(t + 1) * NJ].rearrange(
                                "p (j o) -> p j o", o=1).to_broadcast([128, NJ, BL]),
                            in1=iselb.rearrange("p (o b) -> p o b", o=1).to_broadcast(
                                [128, NJ, BL]),
                            op=OP.mult)

                        # PE: readT (uses X before this step's update)
                        rlo = d_ps.tile([128, BL], F32, tag="rlo", bufs=2)
                        rhi = d_ps.tile([72, BL], F32, tag="rhi", bufs=2)
                        for j in range(NJ):
                            nc.tensor.matmul(rlo, lhsT=x[:, j * DV:j * DV + 128],
                                             rhs=wsel[:, j * BL:(j + 1) * BL],
                                             start=(j == 0), stop=(j == NJ - 1))
                        for j in range(NJ):
                            nc.tensor.matmul(rhi, lhsT=x[:, j * DV + 128:(j + 1) * DV],
                                             rhs=wsel[:, j * BL:(j + 1) * BL],
                                             start=(j == 0), stop=(j == NJ - 1))
                        nc.scalar.copy(readT_lo[:, BL * t:BL * (t + 1)], rlo)
                        nc.scalar.copy(readT_hi[:, BL * t:BL * (t + 1)], rhi[:72, :])

                        # DVE: V = X*er_b ; V -= ad_b (split so ACT starts early)
                        ka = KACT
                        kb = NJ - KACT
                        va = work.tile([128, ka * DV], BF16, tag="va", bufs=2)
                        vb = work.tile([128, kb * DV], BF16, tag="vb", bufs=2)
                        va3 = va.rearrange("p (j d) -> p j d", j=ka)
                        vb3 = vb.rearrange("p (j d) -> p j d", j=kb)
                        x3 = x.rearrange("p (j d) -> p j d", j=NJ)
                        i1 = nc.vector.tensor_tensor(out=va3, in0=x3[:, :ka, :],
                                                     in1=er_b[:, :ka, :], op=OP.mult)
                        i2 = nc.vector.tensor_tensor(out=va3, in0=va3,
                                                     in1=ad_b[:, :ka, :], op=OP.subtract)
                        i3 = nc.vector.tensor_tensor(out=vb3, in0=x3[:, ka:, :],
                                                     in1=er_b[:, ka:, :], op=OP.mult)
                        add_dep_helper(i3.ins, i2.ins, sync=False,
                                       reason="order V_a before V_b")
                        kb0 = kb - (kb % 2)
                        nc.vector.tensor_tensor(out=vb3[:, :kb0, :], in0=vb3[:, :kb0, :],
                                                in1=ad_b[:, ka:ka + kb0, :],
                                                op=OP.subtract)
                        if kb > kb0:
                            nc.vector.tensor_tensor(out=vb3[:, kb0:, :],
                                                    in0=vb3[:, kb0:, :],
                                                    in1=ad_b[:, ka + kb0:, :],
                                                    op=OP.subtract)
                        wg = work.tile([128, FREE], BF16, tag="wg", bufs=2)
                        for j in range(KACT):
                            wcol = w_scan[:, t * NJ + j:t * NJ + j + 1]
                            nc.scalar.activation(out=wg[:, j * DV:(j + 1) * DV],
                                                 in_=va[:, j * DV:(j + 1) * DV],
                                                 func=AF.Copy, scale=wcol)
                        for j in range(KACT, NJ):
                            wcol = w_scan[:, t * NJ + j:t * NJ + j + 1]
                            nc.vector.tensor_scalar(
                                out=wg[:, j * DV:(j + 1) * DV],
                                in0=vb[:, (j - ka) * DV:(j - ka + 1) * DV],
                                scalar1=wcol, scalar2=None, op0=OP.mult)
                        nc.vector.tensor_tensor(out=x, in0=x, in1=wg, op=OP.subtract)

            # ---------- phase E: beta, summ/theta/pred ----------
            with tc.tile_pool(name="e_ps", bufs=2, space="PSUM") as e_ps, \
                 tc.tile_pool(name="e_sb", bufs=1) as e_sb:
                h2T = e_sb.tile([DS + 1, R], BF16)
                nc.sync.dma_start(out=h2T[DS:DS + 1, :], in_=ones_d[:, :R])
                for s_ in range(NSEG):
                    n0, n1 = 512 * s_, min(512 * (s_ + 1), R)
                    hp = e_ps.tile([DS, 512], F32, tag="hp", bufs=2)
                    nc.tensor.matmul(hp[:, :n1 - n0], lhsT=wd1, rhs=qeT[:, n0:n1],
                                     start=True, stop=True)
                    nc.scalar.activation(out=h2T[:DS, n0:n1], in_=hp[:, :n1 - n0],
                                         func=AF.Tanh)
                beta_ps = e_ps.tile([128, NCH], F32, tag="beta_ps", bufs=1)
                for k in range(NCH):
                    nc.tensor.matmul(beta_ps[:, k:k + 1],
                                     lhsT=h2T[:, 128 * k:128 * (k + 1)],
                                     rhs=wd2, start=True, stop=True)
                nc.scalar.copy(beta_sb, beta_ps)
                summT = e_sb.tile([DS + 1, R], BF16)
                nc.sync.dma_start(out=summT[DS:DS + 1, :], in_=ones_d[:, :R])
                for s in range(NSEG):
                    n0, n1 = 512 * s, min(512 * (s + 1), R)
                    sp = e_ps.tile([DS, 512], F32, tag="sp", bufs=2)
                    nc.tensor.matmul(sp[:, :n1 - n0], lhsT=ws1, rhs=readT_lo[:, n0:n1],
                                     start=True, stop=False)
                    nc.tensor.matmul(sp[:, :n1 - n0], lhsT=ws2, rhs=readT_hi[:72, n0:n1],
                                     start=False, stop=False)
                    nc.tensor.matmul(sp[:, :n1 - n0], lhsT=ws3, rhs=qeT[:, n0:n1],
                                     start=False, stop=True)
                    nc.scalar.activation(out=summT[:DS, n0:n1], in_=sp[:, :n1 - n0],
                                         func=AF.Tanh)
                hT = e_sb.tile([DS + 1, R], BF16)
                nc.sync.dma_start(out=hT[DS:DS + 1, :], in_=ones_d[:, :R])
                for s in range(NSEG):
                    n0, n1 = 512 * s, min(512 * (s + 1), R)
                    hp2 = e_ps.tile([DS, 512], F32, tag="hp2", bufs=2)
                    nc.tensor.matmul(hp2[:, :n1 - n0], lhsT=wa1, rhs=summT[:, n0:n1],
                                     start=True, stop=True)
                    nc.scalar.activation(out=hT[:DS, n0:n1], in_=hp2[:, :n1 - n0],
                                         func=AF.Tanh)
                th_ps = e_ps.tile([128, NCH], F32, tag="th_ps", bufs=1)
                for k in range(NCH):
                    nc.tensor.matmul(th_ps[:, k:k + 1], lhsT=hT[:, 128 * k:128 * (k + 1)],
                                     rhs=wa2, start=True, stop=True)
                pre = pers.tile([128, NCH], F32)
                nc.vector.scalar_tensor_tensor(out=pre, in0=th_ps, scalar=SCALE,
                                               in1=beta_sb, op0=OP.mult, op1=OP.subtract)
                pred_sb = pers.tile([128, NCH], F32)
                nc.scalar.activation(out=pred_sb, in_=pre, func=AF.Sigmoid)
                # pred_sb[p=32u+i, k] -> pred_d[b=i, t=4k+u]
                nc.sync.dma_start(
                    out=pred_d.rearrange("b (k u) -> u b k", u=4),
                    in_=pred_sb[:, :])

    return nc


_NC_CACHE = {}


def _get_nc(T=T_FULL):
    if T not in _NC_CACHE:
        n = build_nc(T=T)
        n.compile()
        _NC_CACHE[T] = n
    return _NC_CACHE[T]


def make_inmaps(q_data, qa_data, q_tab, qa_tab, Mk, Mv0, Ws, bs, Wa1, ba1, Wa2,
                ba2, Wd1, bd1, Wd2, bd2, We, be, Wad, bad, T=T_FULL):
    bf = ml_dtypes.bfloat16
    f32 = np.float32
    q_data = np.asarray(q_data)
    qa_data = np.asarray(qa_data)

    mv0p = np.zeros((MP, DV), f32)
    mv0p[:M] = np.asarray(Mv0, f32)
    x0 = mv0p.reshape(4, NJ, DV)[:, None].repeat(BL, 1).reshape(128, FREE).astype(bf)
    isel = np.tile(np.eye(BL, dtype=f32), (4, 1)).astype(bf)  # (128, 32), g-major
    ident = np.eye(128, dtype=f32)

    cat = np.concatenate
    common = {
        "qtab": np.ascontiguousarray(np.asarray(q_tab, f32)),
        "qatab": np.ascontiguousarray(np.asarray(qa_tab, f32)),
        "x0": x0, "isel": isel, "ident": ident,
        "ones": np.ones((1, BL * T), f32).astype(bf),
        "mkt": np.asarray(Mk, f32).T.copy().astype(bf),
        "we1": np.asarray(We, f32)[:128].astype(bf),
        "we2": cat([np.asarray(We, f32)[128:], np.asarray(be, f32)[None, :]], 0).astype(bf),
        "wad1": np.asarray(Wad, f32)[:128].astype(bf),
        "wad2": cat([np.asarray(Wad, f32)[128:], np.asarray(bad, f32)[None, :]], 0).astype(bf),
        "wd1": cat([np.asarray(Wd1, f32), np.asarray(bd1, f32)[None, :]], 0).astype(bf),
        "wd2": cat([np.asarray(Wd2, f32), np.asarray(bd2, f32)[None, :]], 0).astype(bf),
        "ws1": np.asarray(Ws, f32)[:128].astype(bf),
        "ws2": np.asarray(Ws, f32)[128:200].astype(bf),
        "ws3": cat([np.asarray(Ws, f32)[200:], np.asarray(bs, f32)[None, :]], 0).astype(bf),
        "wa1": cat([np.asarray(Wa1, f32), np.asarray(ba1, f32)[None, :]], 0).astype(bf),
        "wa2": cat([np.asarray(Wa2, f32), np.asarray(ba2, f32)[None, :]], 0).astype(bf),
    }
    in_maps = []
    for c in range(NCORES):
        sl = slice(BL * c, BL * (c + 1))
        in_maps.append(dict(
            common,
            qi=np.ascontiguousarray(q_data[sl, :T].T).reshape(-1).astype(np.int32),
            qai=np.ascontiguousarray(qa_data[sl, :T].T).reshape(-1).astype(np.int32)))
    return in_maps


def kernel(**inputs):
    nc = _get_nc(T_FULL)
    in_maps = make_inmaps(**inputs)
    res = run_bass_kernel_spmd(nc, in_maps, core_ids=list(range(NCORES)), trace=False)
    return np.concatenate([res.results[c]["pred"] for c in range(NCORES)], axis=0)
